# revision 12
# baseline (speedup 1.0000x reference)
"""Distributed Trainium2 kernel for nn_Attention_31104153157828.

Computation (B=16, S=2048, D=1024):
    fac1 = k @ W                     [B,S,D]
    fac2 = (q @ U)[:, None, :]       [B,1,D]
    t    = tanh(fac1 + fac2)
    s    = einsum('bsd,bse->bde', v, t)      [B,D,D]
    attn = softmax(s, axis=0)                 (softmax over BATCH)
    out  = einsum('bsd,bde->bse', v, attn)   [B,S,D]

Sharding: data-parallel over batch, 2 batches per core on 8 cores.
The batch-axis softmax needs cross-core AllReduce of max and sum(exp)
over the [D,D] logit matrix (per e-half, bf16 payload).

PE issue rate is ~263ns per 512-row matmul regardless of dtype, so the
schedule minimizes PE instructions and keeps every engine queue free of
head-of-line blocking:
  - k is PE-transposed once per batch (h0 pass); kT is round-tripped
    through DRAM for the h1 pass instead of re-transposing.
  - fac2 is added via a DVE broadcast-add into PSUM, not per-tile K=1
    matmuls.
  - softmax is split into phases (max/AR, exp/AR, rec/mul) and emitted
    interleaved with stage C so AR-gated ops never block C's queue work.
  - 1/Z uses cast + reciprocal_approx_fast (Z >= 1 always).

Emission order:
  A00 B00 A10 B10 | mx0 | A01 | exp0 | B01 | rec0 | A11 B11
  | mx1 | C00 | exp1 | C10 | rec1 | C01 C11
"""
import numpy as np
import concourse.bass as bass
import concourse.bacc as bacc
import concourse.tile as tile
import concourse.mybir as mybir
from concourse.bass_utils import run_bass_kernel_spmd

F32 = mybir.dt.float32
BF16 = mybir.dt.bfloat16
AF = mybir.ActivationFunctionType

B, S, D = 16, 2048, 1024
N_CORES = 8
BL = B // N_CORES          # local batches per core = 2
M_T = S // 128             # 16 s-tiles
KC = D // 128              # 8 contraction chunks (d)
EH = 2                     # e halves of 512
ARC = 4                    # AllReduce chunks (pairs of d-tiles)
RG = [list(range(N_CORES))]


def build():
    nc = bacc.Bacc("TRN2", target_bir_lowering=False, debug=False,
                   num_devices=N_CORES)

    q2 = nc.dram_tensor("q2", [BL, D], F32, kind="ExternalInput")
    k2 = nc.dram_tensor("k2", [BL, S, D], F32, kind="ExternalInput")
    v2 = nc.dram_tensor("v2", [BL, S, D], F32, kind="ExternalInput")
    Wd = nc.dram_tensor("W", [D, D], F32, kind="ExternalInput")
    Ud = nc.dram_tensor("U", [D, D], F32, kind="ExternalInput")
    out2 = nc.dram_tensor("out", [BL, S, D], F32, kind="ExternalOutput")

    # kT bounce (bf16), written during h0 A passes, read during h1
    kt_d = nc.dram_tensor("kt_d", [BL, 128, M_T, KC, 128], BF16)

    # collective bounce buffers, one set per e-half
    mx_in = [nc.dram_tensor(f"mx_in{h}", [128, KC, 512], BF16) for h in range(EH)]
    mx_out = [nc.dram_tensor(f"mx_out{h}", [128, KC, 512], BF16) for h in range(EH)]
    sm_in = [nc.dram_tensor(f"sm_in{h}", [128, KC, 512], BF16) for h in range(EH)]
    sm_out = [nc.dram_tensor(f"sm_out{h}", [128, KC, 512], BF16) for h in range(EH)]

    warm_in = nc.dram_tensor("warm_in", [128, 16], F32)
    warm_out = nc.dram_tensor("warm_out", [128, 16], F32)
    warm_out2 = nc.dram_tensor("warm_out2", [128, 16], F32)

    ident_d = nc.inline_tensor(np.eye(128, dtype=np.float32), name="ident")
    ones_d = nc.inline_tensor(np.ones((1, 128), np.float32), name="ones1")

    with tile.TileContext(nc) as tc:
        with tc.tile_pool(name="rp", bufs=1) as rp:
            ident = rp.tile([128, 128], F32, name="ident_t")
            nc.sync.dma_start(ident[:], ident_d.ap())
            ones_bf = rp.tile([1, 128], BF16, name="ones_bf")
            nc.gpsimd.dma_start(ones_bf[:], ones_d.ap())
            wtile = rp.tile([128, 16], F32, name="wtile")
            nc.gpsimd.dma_start(wtile[:], ident_d.ap()[:, 0:16])
            nc.gpsimd.dma_start(warm_in.ap(), wtile[:])

            # long-lived pools first (LIFO release discipline)
            cp_cm = tc.tile_pool(name="cpool", bufs=2)
            cpool = cp_cm.__enter__()
            vt_cm = tc.tile_pool(name="vtp", bufs=1)
            vtp = vt_cm.__enter__()
            ap_cm = tc.tile_pool(name="attnp", bufs=1)
            attnp = ap_cm.__enter__()
            sm_cm = tc.tile_pool(name="smp", bufs=2)
            smp = sm_cm.__enter__()
            sp_cm = tc.tile_pool(name="spool", bufs=1)
            spool = sp_cm.__enter__()
            wp_cm = tc.tile_pool(name="wp", bufs=1)
            wp = wp_cm.__enter__()

            # W bf16 (half at a time, tag-rotated) + fac2 broadcast tiles
            W_h = {}
            fb = {}

            def load_W_half(h):
                W_h[h] = wp.tile([128, KC, 512], BF16, tag="Wh", name=f"W_h{h}")
                with tc.tile_pool(name=f"wtp{h}", bufs=2) as wtp:
                    for kc in range(KC):
                        wtmp = wtp.tile([128, 512], F32, tag="wtmp",
                                        name=f"wt{h}_{kc}")
                        nc.scalar.dma_start(
                            wtmp[:],
                            Wd.ap().rearrange("(kc p) e -> p kc e", p=128)
                            [:, kc, h * 512:(h + 1) * 512])
                        nc.vector.tensor_copy(W_h[h][:, kc, :], wtmp[:])

            load_W_half(0)

            # fac2 = q @ U -> broadcast tiles fb[(b,h)] = [128,512] bf16
            with (
                tc.tile_pool(name="f2u", bufs=1) as f2u,
                tc.tile_pool(name="f2", bufs=2) as f2p,
                tc.tile_pool(name="f2ps", bufs=2, space="PSUM") as f2ps,
            ):
                U_bf = f2u.tile([128, KC, D], BF16, name="U_bf")
                with tc.tile_pool(name="utp", bufs=2) as utp:
                    for kc in range(KC):
                        utmp = utp.tile([128, D], F32, tag="utmp",
                                        name=f"ut{kc}")
                        nc.scalar.dma_start(
                            utmp[:],
                            Ud.ap().rearrange("(kc p) e -> p kc e", p=128)[:, kc, :])
                        nc.vector.tensor_copy(U_bf[:, kc, :], utmp[:])
                fac2 = f2u.tile([1, BL, D], BF16, name="fac2")
                for b in range(BL):
                    qcol_f = f2p.tile([128, KC], F32, tag="qcf", name=f"qcf{b}")
                    nc.gpsimd.dma_start(
                        qcol_f[:], q2.ap()[b].rearrange("(kc p) -> p kc", p=128))
                    qcol = f2p.tile([128, KC], BF16, tag="qcb", name=f"qcb{b}")
                    nc.vector.tensor_copy(qcol[:], qcol_f[:])
                    for h in range(EH):
                        ps = f2ps.tile([1, 512], F32, tag="f2ps",
                                       name=f"f2ps{b}_{h}")
                        for kc in range(KC):
                            nc.tensor.matmul(ps[:], qcol[:, kc:kc + 1],
                                             U_bf[:, kc, h * 512:(h + 1) * 512],
                                             start=(kc == 0), stop=(kc == KC - 1))
                        nc.scalar.copy(fac2[0:1, b, h * 512:(h + 1) * 512], ps[:])
                # broadcast fac2 across partitions via K=1 matmul
                for b in range(BL):
                    for h in range(EH):
                        psb = f2ps.tile([128, 512], F32, tag="fbps",
                                        name=f"fbps{b}_{h}")
                        nc.tensor.matmul(psb[:], ones_bf[:],
                                         fac2[0:1, b, h * 512:(h + 1) * 512],
                                         start=True, stop=True)
                        fb[(b, h)] = wp.tile([128, 512], BF16, tag=f"fb{b}{h}",
                                             name=f"fb{b}_{h}")
                        nc.vector.tensor_copy(fb[(b, h)][:], psb[:])

            # warm up the collective machinery (after the setup DMAs so the
            # gpsimd queue isn't blocked while the barrier settles)
            ar_w1 = nc.gpsimd.collective_compute(
                "AllReduce", mybir.AluOpType.max, replica_groups=RG,
                ins=[warm_in.ap().opt()], outs=[warm_out.ap().opt()])
            ar_w2 = nc.gpsimd.collective_compute(
                "AllReduce", mybir.AluOpType.add, replica_groups=RG,
                ins=[warm_out.ap().opt()], outs=[warm_out2.ap().opt()])

            # rotating load pools
            vp_cm = tc.tile_pool(name="vp", bufs=2)
            vp = vp_cm.__enter__()
            vbp_cm = tc.tile_pool(name="vbp", bufs=2)
            vbp = vbp_cm.__enter__()
            kp_cm = tc.tile_pool(name="kp", bufs=2)
            kp = kp_cm.__enter__()

            s_cur = [None] * BL
            s_h = {}
            attn = {}
            vT = {0: [], 1: []}

            def stage_A(b, h):
                """returns (t_pool_cm, t_tile); caller's stage_B closes it"""
                t_cm = tc.tile_pool(name=f"t{b}{h}", bufs=1)
                tp = t_cm.__enter__()
                t_bh = tp.tile([128, M_T, 512], BF16, name=f"t{b}_{h}")
                with (
                    tc.tile_pool(name=f"ktm{b}{h}", bufs=2) as ktp,
                    tc.tile_pool(name=f"A{b}{h}ps", bufs=3, space="PSUM") as aps,
                    tc.tile_pool(name=f"A{b}{h}tp", bufs=2, space="PSUM") as tps,
                ):
                    for m in range(M_T):
                        ktm = ktp.tile([128, KC, 128], BF16, tag="ktm",
                                       name=f"ktm{b}_{h}_{m}")
                        if h == 0:
                            kslab = kp.tile([128, D], F32, tag="kslab",
                                            name=f"kslab{b}_{m}")
                            nc.sync.dma_start(
                                kslab[:], k2.ap()[b, m * 128:(m + 1) * 128, :])
                            for g in range(2):
                                ptr = tps.tile([128, 512], F32, tag="ptr",
                                               name=f"ptr{b}_{m}_{g}")
                                for i in range(4):
                                    kc = g * 4 + i
                                    nc.tensor.transpose(
                                        ptr[:, i * 128:(i + 1) * 128],
                                        kslab[:, kc * 128:(kc + 1) * 128],
                                        ident[:])
                                nc.vector.tensor_copy(
                                    ktm[:, g * 4:(g + 1) * 4, :], ptr[:])
                            nc.sync.dma_start(kt_d.ap()[b][:, m, :, :], ktm[:])
                        else:
                            nc.sync.dma_start(ktm[:], kt_d.ap()[b][:, m, :, :])
                        ps = aps.tile([128, 512], F32, tag="aps",
                                      name=f"aps{b}_{h}_{m}")
                        for kc in range(KC):
                            nc.tensor.matmul(
                                ps[:], ktm[:, kc, :], W_h[h][:, kc, :],
                                start=(kc == 0), stop=(kc == KC - 1))
                        nc.vector.tensor_add(ps[:], ps[:], fb[(b, h)][:])
                        nc.scalar.activation(t_bh[:, m, :], ps[:], AF.Tanh)
                return t_cm, t_bh

            def stage_B(b, h, t_cm, t_bh):
                s_t = spool.tile([128, KC, 512], F32, tag=f"s{b}",
                                 name=f"s{b}_{h}")
                s_cur[b] = s_t
                with tc.tile_pool(name=f"B{b}{h}ps", bufs=1,
                                  space="PSUM") as bps:
                    psb = [bps.tile([128, 512], F32, tag=f"pb{dt}",
                                    name=f"pb{b}_{h}_{dt}") for dt in range(KC)]
                    for m in range(M_T):
                        vf = vp.tile([128, D], F32, tag="vf",
                                     name=f"vf{b}_{h}_{m}")
                        if h == 0:
                            nc.scalar.dma_start(
                                vf[:], v2.ap()[b, m * 128:(m + 1) * 128, :])
                        else:
                            nc.sync.dma_start(
                                vf[:], v2.ap()[b, m * 128:(m + 1) * 128, :])
                        vb = vbp.tile([128, D], BF16, tag="vb",
                                      name=f"vbb{b}_{h}_{m}")
                        nc.vector.tensor_copy(vb[:], vf[:])
                        if h == 1:
                            vt = vtp.tile([128, KC, 128], BF16,
                                          tag=f"vt{b}_{m}", name=f"vt{b}_{m}")
                            nc.scalar.dma_start(vt[:], vb[:], transpose=True)
                            vT[b].append(vt)
                        for dt in range(KC):
                            nc.tensor.matmul(
                                psb[dt][:],
                                vb[:, dt * 128:(dt + 1) * 128],
                                t_bh[:, m, :],
                                start=(m == 0), stop=(m == M_T - 1))
                    for dt in range(KC):
                        nc.vector.tensor_copy(s_t[:, dt, :], psb[dt][:])
                t_cm.__exit__(None, None, None)

            prev_ar = [ar_w2]

            def sm_max(h):
                s_h[h] = list(s_cur)
                for b in range(BL):
                    attn[(b, h)] = attnp.tile([128, KC, 512], BF16,
                                              tag=f"at{b}{h}",
                                              name=f"attn{b}_{h}")
                for c in range(ARC):
                    dsl = slice(2 * c, 2 * c + 2)
                    mx = smp.tile([128, 2, 512], BF16, tag="bc1",
                                  name=f"mx{h}_{c}")
                    nc.vector.tensor_max(mx[:], s_h[h][0][:, dsl, :],
                                         s_h[h][1][:, dsl, :])
                    nc.scalar.dma_start(mx_in[h].ap()[:, dsl, :], mx[:])
                ar_mx = nc.gpsimd.collective_compute(
                    "AllReduce", mybir.AluOpType.max, replica_groups=RG,
                    ins=[mx_in[h].ap().opt()], outs=[mx_out[h].ap().opt()])
                tile.add_dep_helper(ar_mx.ins, prev_ar[0].ins, sync=False,
                                    reason="serialize collectives")
                prev_ar[0] = ar_mx

            def sm_exp(h):
                for c in range(ARC):
                    dsl = slice(2 * c, 2 * c + 2)
                    gmxb = smp.tile([128, 2, 512], BF16, tag="bc2",
                                    name=f"gmxb{h}_{c}")
                    nc.gpsimd.dma_start(gmxb[:], mx_out[h].ap()[:, dsl, :])
                    for b in range(BL):
                        nc.vector.tensor_sub(s_h[h][b][:, dsl, :],
                                             s_h[h][b][:, dsl, :], gmxb[:])
                        nc.scalar.activation(attn[(b, h)][:, dsl, :],
                                             s_h[h][b][:, dsl, :], AF.Exp)
                    sm = smp.tile([128, 2, 512], BF16, tag="bc1",
                                  name=f"sm{h}_{c}")
                    nc.vector.tensor_add(sm[:], attn[(0, h)][:, dsl, :],
                                         attn[(1, h)][:, dsl, :])
                    nc.scalar.dma_start(sm_in[h].ap()[:, dsl, :], sm[:])
                ar_sm = nc.gpsimd.collective_compute(
                    "AllReduce", mybir.AluOpType.add, replica_groups=RG,
                    ins=[sm_in[h].ap().opt()], outs=[sm_out[h].ap().opt()])
                tile.add_dep_helper(ar_sm.ins, prev_ar[0].ins, sync=False,
                                    reason="serialize collectives")
                prev_ar[0] = ar_sm

            def sm_rec(h):
                # rec = 1/Z via fast approx (Z >= 1, so no edge cases);
                # attn = p * rec in place (bf16)
                for c in range(2 * ARC):
                    dsl = slice(c, c + 1)
                    zz = smp.tile([128, 1, 512], BF16, tag="bc2",
                                  name=f"zz{h}_{c}")
                    nc.gpsimd.dma_start(zz[:], sm_out[h].ap()[:, dsl, :])
                    zf = smp.tile([128, 1, 512], F32, tag="zf",
                                  name=f"zf{h}_{c}")
                    nc.vector.tensor_copy(zf[:], zz[:])
                    rec = smp.tile([128, 1, 512], F32, tag="rec",
                                   name=f"rec{h}_{c}")
                    nc.vector.reciprocal_approx_fast(rec[:], zf[:])
                    for b in range(BL):
                        nc.vector.tensor_mul(attn[(b, h)][:, dsl, :],
                                             attn[(b, h)][:, dsl, :], rec[:])

            # ======== main schedule ========
            t_cm, t_bh = stage_A(0, 0)
            stage_B(0, 0, t_cm, t_bh)
            t_cm, t_bh = stage_A(1, 0)
            load_W_half(1)          # rotate W to the h1 half during A10/B10
            stage_B(1, 0, t_cm, t_bh)

            sm_max(0)
            t_cm, t_bh = stage_A(0, 1)
            sm_exp(0)
            stage_B(0, 1, t_cm, t_bh)
            t_cm, t_bh = stage_A(1, 1)
            sm_rec(0)
            stage_B(1, 1, t_cm, t_bh)

            kp_cm.__exit__(None, None, None)
            vbp_cm.__exit__(None, None, None)
            vp_cm.__exit__(None, None, None)

            sm_max(1)

            # ======== stage C (interleaved with softmax h1 tail) ========
            cps_cm = tc.tile_pool(name="cps", bufs=3, space="PSUM")
            cps = cps_cm.__enter__()

            def stage_c(b, h):
                he = slice(h * 512, (h + 1) * 512)
                for m in range(M_T):
                    ps = cps.tile([128, 512], F32, tag="cps",
                                  name=f"cps{b}_{h}_{m}")
                    for kc in range(KC):
                        nc.tensor.matmul(
                            ps[:], vT[b][m][:, kc, :],
                            attn[(b, h)][:, kc, :],
                            start=(kc == 0), stop=(kc == KC - 1))
                    ost = cpool.tile([128, 512], F32, tag="ost",
                                     name=f"ost{b}_{h}_{m}")
                    nc.scalar.copy(ost[:], ps[:])
                    nc.sync.dma_start(
                        out2.ap()[b, m * 128:(m + 1) * 128, he], ost[:])

            stage_c(0, 0)
            sm_exp(1)
            stage_c(1, 0)
            sm_rec(1)

            wp_cm.__exit__(None, None, None)
            sp_cm.__exit__(None, None, None)
            sm_cm.__exit__(None, None, None)

            stage_c(0, 1)
            stage_c(1, 1)

            cps_cm.__exit__(None, None, None)
            ap_cm.__exit__(None, None, None)
            vt_cm.__exit__(None, None, None)
            cp_cm.__exit__(None, None, None)

    nc.compile()
    return nc


_NC = None


def _get_nc():
    global _NC
    if _NC is None:
        _NC = build()
    return _NC


def kernel(q, k, v, W, U):
    q = np.ascontiguousarray(np.asarray(q, dtype=np.float32))
    k = np.ascontiguousarray(np.asarray(k, dtype=np.float32))
    v = np.ascontiguousarray(np.asarray(v, dtype=np.float32))
    W = np.ascontiguousarray(np.asarray(W, dtype=np.float32))
    U = np.ascontiguousarray(np.asarray(U, dtype=np.float32))

    nc = _get_nc()
    in_maps = [
        {
            "q2": q[c * BL:(c + 1) * BL],
            "k2": k[c * BL:(c + 1) * BL],
            "v2": v[c * BL:(c + 1) * BL],
            "W": W,
            "U": U,
        }
        for c in range(N_CORES)
    ]
    res = run_bass_kernel_spmd(nc, in_maps, core_ids=list(range(N_CORES)))
    out = np.concatenate([res.results[c]["out"] for c in range(N_CORES)], axis=0)
    return out.astype(np.float32)


if __name__ == "__main__":
    rng = np.random.default_rng(0)
    q = rng.standard_normal((B, D), dtype=np.float32)
    k = rng.standard_normal((B, S, D), dtype=np.float32)
    v = rng.standard_normal((B, S, D), dtype=np.float32)
    W = (rng.standard_normal((D, D), dtype=np.float32) / np.sqrt(D)).astype(np.float32)
    U = (rng.standard_normal((D, D), dtype=np.float32) / np.sqrt(D)).astype(np.float32)
    out = kernel(q=q, k=k, v=v, W=W, U=U)
    print("out", out.shape, out.dtype, float(np.abs(out).mean()))


# revision 14
# speedup vs baseline: 1.0377x; 1.0377x over previous
"""Distributed Trainium2 kernel for nn_Attention_31104153157828.

Computation (B=16, S=2048, D=1024):
    fac1 = k @ W                     [B,S,D]
    fac2 = (q @ U)[:, None, :]       [B,1,D]
    t    = tanh(fac1 + fac2)
    s    = einsum('bsd,bse->bde', v, t)      [B,D,D]
    attn = softmax(s, axis=0)                 (softmax over BATCH)
    out  = einsum('bsd,bde->bse', v, attn)   [B,S,D]

Sharding: data-parallel over batch, 2 batches per core on 8 cores.
The batch-axis softmax needs cross-core AllReduce of max and sum(exp)
over the [D,D] logit matrix (per e-half, bf16 payload).

PE issue rate is ~263ns per 512-row matmul regardless of dtype, so the
schedule minimizes PE instructions and keeps every engine queue free of
head-of-line blocking:
  - k is PE-transposed once per batch (h0 pass); kT is round-tripped
    through DRAM for the h1 pass instead of re-transposing.
  - fac2 is added via a DVE broadcast-add into PSUM, not per-tile K=1
    matmuls.
  - softmax is split into phases (max/AR, exp/AR, rec/mul) and emitted
    interleaved with stage C so AR-gated ops never block C's queue work.
  - 1/Z uses cast + reciprocal_approx_fast (Z >= 1 always).

Emission order:
  A00 B00 A10 B10 | mx0 | A01 | exp0 | B01 | rec0 | A11 B11
  | mx1 | C00 | exp1 | C10 | rec1 | C01 C11
"""
import numpy as np
import concourse.bass as bass
import concourse.bacc as bacc
import concourse.tile as tile
import concourse.mybir as mybir
from concourse.bass_utils import run_bass_kernel_spmd

F32 = mybir.dt.float32
BF16 = mybir.dt.bfloat16
AF = mybir.ActivationFunctionType

B, S, D = 16, 2048, 1024
N_CORES = 8
BL = B // N_CORES          # local batches per core = 2
M_T = S // 128             # 16 s-tiles
KC = D // 128              # 8 contraction chunks (d)
EH = 2                     # e halves of 512
ARC = 4                    # AllReduce chunks (pairs of d-tiles)
RG = [list(range(N_CORES))]


def build():
    nc = bacc.Bacc("TRN2", target_bir_lowering=False, debug=False,
                   num_devices=N_CORES)

    q2 = nc.dram_tensor("q2", [BL, D], F32, kind="ExternalInput")
    k2 = nc.dram_tensor("k2", [BL, S, D], F32, kind="ExternalInput")
    v2 = nc.dram_tensor("v2", [BL, S, D], F32, kind="ExternalInput")
    Wd = nc.dram_tensor("W", [D, D], F32, kind="ExternalInput")
    Ud = nc.dram_tensor("U", [D, D], F32, kind="ExternalInput")
    out2 = nc.dram_tensor("out", [BL, S, D], F32, kind="ExternalOutput")

    # kT bounce (bf16), written during h0 A passes, read during h1
    kt_d = nc.dram_tensor("kt_d", [BL, 128, M_T, KC, 128], BF16)

    # collective bounce buffers, one set per e-half
    mx_in = [nc.dram_tensor(f"mx_in{h}", [128, KC, 512], BF16) for h in range(EH)]
    mx_out = [nc.dram_tensor(f"mx_out{h}", [128, KC, 512], BF16) for h in range(EH)]
    sm_in = [nc.dram_tensor(f"sm_in{h}", [128, KC, 512], BF16) for h in range(EH)]
    sm_out = [nc.dram_tensor(f"sm_out{h}", [128, KC, 512], BF16) for h in range(EH)]

    warm_in = nc.dram_tensor("warm_in", [128, 16], F32)
    warm_out = nc.dram_tensor("warm_out", [128, 16], F32)
    warm_out2 = nc.dram_tensor("warm_out2", [128, 16], F32)

    ident_d = nc.inline_tensor(np.eye(128, dtype=np.float32), name="ident")
    ones_d = nc.inline_tensor(np.ones((1, 128), np.float32), name="ones1")

    with tile.TileContext(nc) as tc:
        with tc.tile_pool(name="rp", bufs=1) as rp:
            ident = rp.tile([128, 128], F32, name="ident_t")
            nc.sync.dma_start(ident[:], ident_d.ap())
            ones_bf = rp.tile([1, 128], BF16, name="ones_bf")
            nc.gpsimd.dma_start(ones_bf[:], ones_d.ap())
            wtile = rp.tile([128, 16], F32, name="wtile")
            nc.gpsimd.dma_start(wtile[:], ident_d.ap()[:, 0:16])
            nc.gpsimd.dma_start(warm_in.ap(), wtile[:])

            # long-lived pools first (LIFO release discipline)
            cp_cm = tc.tile_pool(name="cpool", bufs=2)
            cpool = cp_cm.__enter__()
            vt_cm = tc.tile_pool(name="vtp", bufs=1)
            vtp = vt_cm.__enter__()
            ap_cm = tc.tile_pool(name="attnp", bufs=1)
            attnp = ap_cm.__enter__()
            sm_cm = tc.tile_pool(name="smp", bufs=2)
            smp = sm_cm.__enter__()
            sp_cm = tc.tile_pool(name="spool", bufs=1)
            spool = sp_cm.__enter__()
            wp_cm = tc.tile_pool(name="wp", bufs=1)
            wp = wp_cm.__enter__()

            # W bf16 (half at a time, tag-rotated) + fac2 broadcast tiles
            W_h = {}
            fb = {}

            def load_W_half(h):
                W_h[h] = wp.tile([128, KC, 512], BF16, tag="Wh", name=f"W_h{h}")
                with tc.tile_pool(name=f"wtp{h}", bufs=2) as wtp:
                    for kc in range(KC):
                        wtmp = wtp.tile([128, 512], F32, tag="wtmp",
                                        name=f"wt{h}_{kc}")
                        nc.scalar.dma_start(
                            wtmp[:],
                            Wd.ap().rearrange("(kc p) e -> p kc e", p=128)
                            [:, kc, h * 512:(h + 1) * 512])
                        nc.vector.tensor_copy(W_h[h][:, kc, :], wtmp[:])

            # fac2 = q @ U -> broadcast tiles fb[(b,h)] = [128,512] bf16
            # (U first: the fb chain is the longest pole for A00's first tanh)
            with (
                tc.tile_pool(name="f2u", bufs=1) as f2u,
                tc.tile_pool(name="f2", bufs=2) as f2p,
                tc.tile_pool(name="f2ps", bufs=2, space="PSUM") as f2ps,
            ):
                U_bf = f2u.tile([128, KC, D], BF16, name="U_bf")
                with tc.tile_pool(name="utp", bufs=2) as utp:
                    for kc in range(KC):
                        utmp = utp.tile([128, D], F32, tag="utmp",
                                        name=f"ut{kc}")
                        nc.scalar.dma_start(
                            utmp[:],
                            Ud.ap().rearrange("(kc p) e -> p kc e", p=128)[:, kc, :])
                        nc.vector.tensor_copy(U_bf[:, kc, :], utmp[:])
                fac2 = f2u.tile([1, BL, D], BF16, name="fac2")
                for b in range(BL):
                    qcol_f = f2p.tile([128, KC], F32, tag="qcf", name=f"qcf{b}")
                    nc.gpsimd.dma_start(
                        qcol_f[:], q2.ap()[b].rearrange("(kc p) -> p kc", p=128))
                    qcol = f2p.tile([128, KC], BF16, tag="qcb", name=f"qcb{b}")
                    nc.vector.tensor_copy(qcol[:], qcol_f[:])
                    for h in range(EH):
                        ps = f2ps.tile([1, 512], F32, tag="f2ps",
                                       name=f"f2ps{b}_{h}")
                        for kc in range(KC):
                            nc.tensor.matmul(ps[:], qcol[:, kc:kc + 1],
                                             U_bf[:, kc, h * 512:(h + 1) * 512],
                                             start=(kc == 0), stop=(kc == KC - 1))
                        nc.scalar.copy(fac2[0:1, b, h * 512:(h + 1) * 512], ps[:])
                # broadcast fac2 across partitions via K=1 matmul
                for b in range(BL):
                    for h in range(EH):
                        psb = f2ps.tile([128, 512], F32, tag="fbps",
                                        name=f"fbps{b}_{h}")
                        nc.tensor.matmul(psb[:], ones_bf[:],
                                         fac2[0:1, b, h * 512:(h + 1) * 512],
                                         start=True, stop=True)
                        fb[(b, h)] = wp.tile([128, 512], BF16, tag=f"fb{b}{h}",
                                             name=f"fb{b}_{h}")
                        nc.vector.tensor_copy(fb[(b, h)][:], psb[:])

            load_W_half(0)

            # warm up the collective machinery (after the setup DMAs so the
            # gpsimd queue isn't blocked while the barrier settles)
            ar_w1 = nc.gpsimd.collective_compute(
                "AllReduce", mybir.AluOpType.max, replica_groups=RG,
                ins=[warm_in.ap().opt()], outs=[warm_out.ap().opt()])
            ar_w2 = nc.gpsimd.collective_compute(
                "AllReduce", mybir.AluOpType.add, replica_groups=RG,
                ins=[warm_out.ap().opt()], outs=[warm_out2.ap().opt()])

            # rotating load pools
            vp_cm = tc.tile_pool(name="vp", bufs=2)
            vp = vp_cm.__enter__()
            vbp_cm = tc.tile_pool(name="vbp", bufs=2)
            vbp = vbp_cm.__enter__()
            kp_cm = tc.tile_pool(name="kp", bufs=2)
            kp = kp_cm.__enter__()

            s_cur = [None] * BL
            s_h = {}
            attn = {}
            vT = {0: [], 1: []}

            def stage_A(b, h):
                """returns (t_pool_cm, t_tile); caller's stage_B closes it"""
                t_cm = tc.tile_pool(name=f"t{b}{h}", bufs=1)
                tp = t_cm.__enter__()
                t_bh = tp.tile([128, M_T, 512], BF16, name=f"t{b}_{h}")
                with (
                    tc.tile_pool(name=f"ktm{b}{h}", bufs=2) as ktp,
                    tc.tile_pool(name=f"A{b}{h}ps", bufs=3, space="PSUM") as aps,
                    tc.tile_pool(name=f"A{b}{h}tp", bufs=2, space="PSUM") as tps,
                ):
                    for m in range(M_T):
                        ktm = ktp.tile([128, KC, 128], BF16, tag="ktm",
                                       name=f"ktm{b}_{h}_{m}")
                        if h == 0:
                            kslab = kp.tile([128, D], F32, tag="kslab",
                                            name=f"kslab{b}_{m}")
                            nc.sync.dma_start(
                                kslab[:], k2.ap()[b, m * 128:(m + 1) * 128, :])
                            for g in range(2):
                                ptr = tps.tile([128, 512], F32, tag="ptr",
                                               name=f"ptr{b}_{m}_{g}")
                                for i in range(4):
                                    kc = g * 4 + i
                                    nc.tensor.transpose(
                                        ptr[:, i * 128:(i + 1) * 128],
                                        kslab[:, kc * 128:(kc + 1) * 128],
                                        ident[:])
                                nc.vector.tensor_copy(
                                    ktm[:, g * 4:(g + 1) * 4, :], ptr[:])
                            nc.sync.dma_start(kt_d.ap()[b][:, m, :, :], ktm[:])
                        else:
                            nc.sync.dma_start(ktm[:], kt_d.ap()[b][:, m, :, :])
                        ps = aps.tile([128, 512], F32, tag="aps",
                                      name=f"aps{b}_{h}_{m}")
                        for kc in range(KC):
                            nc.tensor.matmul(
                                ps[:], ktm[:, kc, :], W_h[h][:, kc, :],
                                start=(kc == 0), stop=(kc == KC - 1))
                        nc.vector.tensor_add(ps[:], ps[:], fb[(b, h)][:])
                        nc.scalar.activation(t_bh[:, m, :], ps[:], AF.Tanh)
                return t_cm, t_bh

            def stage_B(b, h, t_cm, t_bh):
                s_t = spool.tile([128, KC, 512], F32, tag=f"s{b}",
                                 name=f"s{b}_{h}")
                s_cur[b] = s_t
                with tc.tile_pool(name=f"B{b}{h}ps", bufs=1,
                                  space="PSUM") as bps:
                    psb = [bps.tile([128, 512], F32, tag=f"pb{dt}",
                                    name=f"pb{b}_{h}_{dt}") for dt in range(KC)]
                    for m in range(M_T):
                        vf = vp.tile([128, D], F32, tag="vf",
                                     name=f"vf{b}_{h}_{m}")
                        if h == 0:
                            nc.scalar.dma_start(
                                vf[:], v2.ap()[b, m * 128:(m + 1) * 128, :])
                        else:
                            nc.sync.dma_start(
                                vf[:], v2.ap()[b, m * 128:(m + 1) * 128, :])
                        vb = vbp.tile([128, D], BF16, tag="vb",
                                      name=f"vbb{b}_{h}_{m}")
                        nc.vector.tensor_copy(vb[:], vf[:])
                        if h == 1:
                            vt = vtp.tile([128, KC, 128], BF16,
                                          tag=f"vt{b}_{m}", name=f"vt{b}_{m}")
                            nc.scalar.dma_start(vt[:], vb[:], transpose=True)
                            vT[b].append(vt)
                        for dt in range(KC):
                            nc.tensor.matmul(
                                psb[dt][:],
                                vb[:, dt * 128:(dt + 1) * 128],
                                t_bh[:, m, :],
                                start=(m == 0), stop=(m == M_T - 1))
                    for dt in range(KC):
                        nc.vector.tensor_copy(s_t[:, dt, :], psb[dt][:])
                t_cm.__exit__(None, None, None)

            prev_ar = [ar_w2]

            def sm_max(h):
                s_h[h] = list(s_cur)
                for b in range(BL):
                    attn[(b, h)] = attnp.tile([128, KC, 512], BF16,
                                              tag=f"at{b}{h}",
                                              name=f"attn{b}_{h}")
                for c in range(ARC):
                    dsl = slice(2 * c, 2 * c + 2)
                    mx = smp.tile([128, 2, 512], BF16, tag="bc1",
                                  name=f"mx{h}_{c}")
                    nc.vector.tensor_max(mx[:], s_h[h][0][:, dsl, :],
                                         s_h[h][1][:, dsl, :])
                    nc.scalar.dma_start(mx_in[h].ap()[:, dsl, :], mx[:])
                ar_mx = nc.gpsimd.collective_compute(
                    "AllReduce", mybir.AluOpType.max, replica_groups=RG,
                    ins=[mx_in[h].ap().opt()], outs=[mx_out[h].ap().opt()])
                tile.add_dep_helper(ar_mx.ins, prev_ar[0].ins, sync=False,
                                    reason="serialize collectives")
                prev_ar[0] = ar_mx

            def sm_exp(h, eng):
                for c in range(ARC):
                    dsl = slice(2 * c, 2 * c + 2)
                    gmxb = smp.tile([128, 2, 512], BF16, tag="bc2",
                                    name=f"gmxb{h}_{c}")
                    nc.gpsimd.dma_start(gmxb[:], mx_out[h].ap()[:, dsl, :])
                    for b in range(BL):
                        eng.tensor_sub(s_h[h][b][:, dsl, :],
                                       s_h[h][b][:, dsl, :], gmxb[:])
                        nc.scalar.activation(attn[(b, h)][:, dsl, :],
                                             s_h[h][b][:, dsl, :], AF.Exp)
                    sm = smp.tile([128, 2, 512], BF16, tag="bc1",
                                  name=f"sm{h}_{c}")
                    eng.tensor_add(sm[:], attn[(0, h)][:, dsl, :],
                                   attn[(1, h)][:, dsl, :])
                    nc.scalar.dma_start(sm_in[h].ap()[:, dsl, :], sm[:])
                ar_sm = nc.gpsimd.collective_compute(
                    "AllReduce", mybir.AluOpType.add, replica_groups=RG,
                    ins=[sm_in[h].ap().opt()], outs=[sm_out[h].ap().opt()])
                tile.add_dep_helper(ar_sm.ins, prev_ar[0].ins, sync=False,
                                    reason="serialize collectives")
                prev_ar[0] = ar_sm

            def sm_rec(h):
                # rec = 1/Z via fast approx (Z >= 1, so no edge cases);
                # attn = p * rec in place (bf16)
                for c in range(2 * ARC):
                    dsl = slice(c, c + 1)
                    zz = smp.tile([128, 1, 512], BF16, tag="bc2",
                                  name=f"zz{h}_{c}")
                    nc.gpsimd.dma_start(zz[:], sm_out[h].ap()[:, dsl, :])
                    zf = smp.tile([128, 1, 512], F32, tag="zf",
                                  name=f"zf{h}_{c}")
                    nc.vector.tensor_copy(zf[:], zz[:])
                    rec = smp.tile([128, 1, 512], F32, tag="rec",
                                   name=f"rec{h}_{c}")
                    nc.vector.reciprocal_approx_fast(rec[:], zf[:])
                    for b in range(BL):
                        nc.vector.tensor_mul(attn[(b, h)][:, dsl, :],
                                             attn[(b, h)][:, dsl, :], rec[:])

            # ======== main schedule ========
            t_cm, t_bh = stage_A(0, 0)
            stage_B(0, 0, t_cm, t_bh)
            t_cm, t_bh = stage_A(1, 0)
            load_W_half(1)          # rotate W to the h1 half during A10/B10
            stage_B(1, 0, t_cm, t_bh)

            sm_max(0)
            t_cm, t_bh = stage_A(0, 1)
            sm_exp(0, nc.gpsimd)
            stage_B(0, 1, t_cm, t_bh)
            t_cm, t_bh = stage_A(1, 1)
            stage_B(1, 1, t_cm, t_bh)
            sm_rec(0)

            kp_cm.__exit__(None, None, None)
            vbp_cm.__exit__(None, None, None)
            vp_cm.__exit__(None, None, None)

            sm_max(1)

            # ======== stage C (interleaved with softmax h1 tail) ========
            cps_cm = tc.tile_pool(name="cps", bufs=3, space="PSUM")
            cps = cps_cm.__enter__()

            def stage_c(b, h):
                he = slice(h * 512, (h + 1) * 512)
                for m in range(M_T):
                    ps = cps.tile([128, 512], F32, tag="cps",
                                  name=f"cps{b}_{h}_{m}")
                    for kc in range(KC):
                        nc.tensor.matmul(
                            ps[:], vT[b][m][:, kc, :],
                            attn[(b, h)][:, kc, :],
                            start=(kc == 0), stop=(kc == KC - 1))
                    ost = cpool.tile([128, 512], F32, tag="ost",
                                     name=f"ost{b}_{h}_{m}")
                    nc.scalar.copy(ost[:], ps[:])
                    nc.sync.dma_start(
                        out2.ap()[b, m * 128:(m + 1) * 128, he], ost[:])

            stage_c(0, 0)
            sm_exp(1, nc.vector)
            stage_c(1, 0)
            sm_rec(1)

            wp_cm.__exit__(None, None, None)
            sp_cm.__exit__(None, None, None)
            sm_cm.__exit__(None, None, None)

            stage_c(0, 1)
            stage_c(1, 1)

            cps_cm.__exit__(None, None, None)
            ap_cm.__exit__(None, None, None)
            vt_cm.__exit__(None, None, None)
            cp_cm.__exit__(None, None, None)

    nc.compile()
    return nc


_NC = None


def _get_nc():
    global _NC
    if _NC is None:
        _NC = build()
    return _NC


def kernel(q, k, v, W, U):
    q = np.ascontiguousarray(np.asarray(q, dtype=np.float32))
    k = np.ascontiguousarray(np.asarray(k, dtype=np.float32))
    v = np.ascontiguousarray(np.asarray(v, dtype=np.float32))
    W = np.ascontiguousarray(np.asarray(W, dtype=np.float32))
    U = np.ascontiguousarray(np.asarray(U, dtype=np.float32))

    nc = _get_nc()
    in_maps = [
        {
            "q2": q[c * BL:(c + 1) * BL],
            "k2": k[c * BL:(c + 1) * BL],
            "v2": v[c * BL:(c + 1) * BL],
            "W": W,
            "U": U,
        }
        for c in range(N_CORES)
    ]
    res = run_bass_kernel_spmd(nc, in_maps, core_ids=list(range(N_CORES)))
    out = np.concatenate([res.results[c]["out"] for c in range(N_CORES)], axis=0)
    return out.astype(np.float32)


if __name__ == "__main__":
    rng = np.random.default_rng(0)
    q = rng.standard_normal((B, D), dtype=np.float32)
    k = rng.standard_normal((B, S, D), dtype=np.float32)
    v = rng.standard_normal((B, S, D), dtype=np.float32)
    W = (rng.standard_normal((D, D), dtype=np.float32) / np.sqrt(D)).astype(np.float32)
    U = (rng.standard_normal((D, D), dtype=np.float32) / np.sqrt(D)).astype(np.float32)
    out = kernel(q=q, k=k, v=v, W=W, U=U)
    print("out", out.shape, out.dtype, float(np.abs(out).mean()))


# revision 15
# speedup vs baseline: 1.0640x; 1.0253x over previous
"""Distributed Trainium2 kernel for nn_Attention_31104153157828.

Computation (B=16, S=2048, D=1024):
    fac1 = k @ W                     [B,S,D]
    fac2 = (q @ U)[:, None, :]       [B,1,D]
    t    = tanh(fac1 + fac2)
    s    = einsum('bsd,bse->bde', v, t)      [B,D,D]
    attn = softmax(s, axis=0)                 (softmax over BATCH)
    out  = einsum('bsd,bde->bse', v, attn)   [B,S,D]

Sharding: data-parallel over batch, 2 batches per core on 8 cores.
The batch-axis softmax needs cross-core AllReduce of max and sum(exp)
over the [D,D] logit matrix (per e-half, bf16 payload).

PE issue rate is ~263ns per 512-row matmul regardless of dtype, so the
schedule minimizes PE instructions and keeps every engine queue free of
head-of-line blocking:
  - k is PE-transposed once per batch (h0 pass); kT is round-tripped
    through DRAM for the h1 pass instead of re-transposing.
  - fac2 is added via a DVE broadcast-add into PSUM, not per-tile K=1
    matmuls.
  - softmax is split into phases (max/AR, exp/AR, rec/mul) and emitted
    interleaved with stage C so AR-gated ops never block C's queue work.
  - 1/Z uses cast + reciprocal_approx_fast (Z >= 1 always).

Emission order:
  A00 B00 A10 B10 | mx0 | A01 | exp0 | B01 | rec0 | A11 B11
  | mx1 | C00 | exp1 | C10 | rec1 | C01 C11
"""
import numpy as np
import concourse.bass as bass
import concourse.bacc as bacc
import concourse.tile as tile
import concourse.mybir as mybir
from concourse.bass_utils import run_bass_kernel_spmd

F32 = mybir.dt.float32
BF16 = mybir.dt.bfloat16
AF = mybir.ActivationFunctionType

B, S, D = 16, 2048, 1024
N_CORES = 8
BL = B // N_CORES          # local batches per core = 2
M_T = S // 128             # 16 s-tiles
KC = D // 128              # 8 contraction chunks (d)
EH = 2                     # e halves of 512
ARC = 4                    # AllReduce chunks (pairs of d-tiles)
RG = [list(range(N_CORES))]


def build():
    nc = bacc.Bacc("TRN2", target_bir_lowering=False, debug=False,
                   num_devices=N_CORES)

    q2 = nc.dram_tensor("q2", [BL, D], F32, kind="ExternalInput")
    k2 = nc.dram_tensor("k2", [BL, S, D], F32, kind="ExternalInput")
    v2 = nc.dram_tensor("v2", [BL, S, D], F32, kind="ExternalInput")
    Wd = nc.dram_tensor("W", [D, D], F32, kind="ExternalInput")
    Ud = nc.dram_tensor("U", [D, D], F32, kind="ExternalInput")
    out2 = nc.dram_tensor("out", [BL, S, D], F32, kind="ExternalOutput")

    # kT bounce (bf16), written during h0 A passes, read during h1
    kt_d = nc.dram_tensor("kt_d", [BL, 128, M_T, KC, 128], BF16)

    # collective bounce buffers, one set per e-half
    mx_in = [nc.dram_tensor(f"mx_in{h}", [128, KC, 512], BF16) for h in range(EH)]
    mx_out = [nc.dram_tensor(f"mx_out{h}", [128, KC, 512], BF16) for h in range(EH)]
    sm_in = [nc.dram_tensor(f"sm_in{h}", [128, KC, 512], BF16) for h in range(EH)]
    sm_out = [nc.dram_tensor(f"sm_out{h}", [128, KC, 512], BF16) for h in range(EH)]

    warm_in = nc.dram_tensor("warm_in", [128, 16], F32)
    warm_out = nc.dram_tensor("warm_out", [128, 16], F32)
    warm_out2 = nc.dram_tensor("warm_out2", [128, 16], F32)

    ident_d = nc.inline_tensor(np.eye(128, dtype=np.float32), name="ident")
    ones_d = nc.inline_tensor(np.ones((1, 128), np.float32), name="ones1")

    with tile.TileContext(nc) as tc:
        with tc.tile_pool(name="rp", bufs=1) as rp:
            ident = rp.tile([128, 128], F32, name="ident_t")
            nc.sync.dma_start(ident[:], ident_d.ap())
            ones_bf = rp.tile([1, 128], BF16, name="ones_bf")
            nc.gpsimd.dma_start(ones_bf[:], ones_d.ap())
            wtile = rp.tile([128, 16], F32, name="wtile")
            nc.gpsimd.dma_start(wtile[:], ident_d.ap()[:, 0:16])
            nc.gpsimd.dma_start(warm_in.ap(), wtile[:])

            # long-lived pools first (LIFO release discipline)
            cp_cm = tc.tile_pool(name="cpool", bufs=2)
            cpool = cp_cm.__enter__()
            vt_cm = tc.tile_pool(name="vtp", bufs=1)
            vtp = vt_cm.__enter__()
            ap_cm = tc.tile_pool(name="attnp", bufs=1)
            attnp = ap_cm.__enter__()
            sm_cm = tc.tile_pool(name="smp", bufs=2)
            smp = sm_cm.__enter__()
            sp_cm = tc.tile_pool(name="spool", bufs=1)
            spool = sp_cm.__enter__()
            wp_cm = tc.tile_pool(name="wp", bufs=1)
            wp = wp_cm.__enter__()

            # W bf16 (half at a time, tag-rotated) + fac2 broadcast tiles
            W_h = {}
            fb = {}

            def load_W_half(h):
                W_h[h] = wp.tile([128, KC, 512], BF16, tag="Wh", name=f"W_h{h}")
                with tc.tile_pool(name=f"wtp{h}", bufs=2) as wtp:
                    for kc in range(KC):
                        wtmp = wtp.tile([128, 512], F32, tag="wtmp",
                                        name=f"wt{h}_{kc}")
                        nc.scalar.dma_start(
                            wtmp[:],
                            Wd.ap().rearrange("(kc p) e -> p kc e", p=128)
                            [:, kc, h * 512:(h + 1) * 512])
                        nc.vector.tensor_copy(W_h[h][:, kc, :], wtmp[:])

            # fac2 = q @ U -> broadcast tiles fb[(b,h)] = [128,512] bf16
            # (U first: the fb chain is the longest pole for A00's first tanh)
            with (
                tc.tile_pool(name="f2u", bufs=1) as f2u,
                tc.tile_pool(name="f2", bufs=2) as f2p,
                tc.tile_pool(name="f2ps", bufs=2, space="PSUM") as f2ps,
            ):
                U_bf = f2u.tile([128, KC, D], BF16, name="U_bf")
                with tc.tile_pool(name="utp", bufs=2) as utp:
                    for kc in range(KC):
                        utmp = utp.tile([128, D], F32, tag="utmp",
                                        name=f"ut{kc}")
                        nc.scalar.dma_start(
                            utmp[:],
                            Ud.ap().rearrange("(kc p) e -> p kc e", p=128)[:, kc, :])
                        nc.vector.tensor_copy(U_bf[:, kc, :], utmp[:])
                fac2 = f2u.tile([1, BL, D], BF16, name="fac2")
                for b in range(BL):
                    qcol_f = f2p.tile([128, KC], F32, tag="qcf", name=f"qcf{b}")
                    nc.gpsimd.dma_start(
                        qcol_f[:], q2.ap()[b].rearrange("(kc p) -> p kc", p=128))
                    qcol = f2p.tile([128, KC], BF16, tag="qcb", name=f"qcb{b}")
                    nc.vector.tensor_copy(qcol[:], qcol_f[:])
                    for h in range(EH):
                        ps = f2ps.tile([1, 512], F32, tag="f2ps",
                                       name=f"f2ps{b}_{h}")
                        for kc in range(KC):
                            nc.tensor.matmul(ps[:], qcol[:, kc:kc + 1],
                                             U_bf[:, kc, h * 512:(h + 1) * 512],
                                             start=(kc == 0), stop=(kc == KC - 1))
                        nc.scalar.copy(fac2[0:1, b, h * 512:(h + 1) * 512], ps[:])
                # broadcast fac2 across partitions via K=1 matmul
                for b in range(BL):
                    for h in range(EH):
                        psb = f2ps.tile([128, 512], F32, tag="fbps",
                                        name=f"fbps{b}_{h}")
                        nc.tensor.matmul(psb[:], ones_bf[:],
                                         fac2[0:1, b, h * 512:(h + 1) * 512],
                                         start=True, stop=True)
                        fb[(b, h)] = wp.tile([128, 512], BF16, tag=f"fb{b}{h}",
                                             name=f"fb{b}_{h}")
                        nc.vector.tensor_copy(fb[(b, h)][:], psb[:])

            load_W_half(0)

            # warm up the collective machinery (after the setup DMAs so the
            # gpsimd queue isn't blocked while the barrier settles)
            ar_w1 = nc.gpsimd.collective_compute(
                "AllReduce", mybir.AluOpType.max, replica_groups=RG,
                ins=[warm_in.ap().opt()], outs=[warm_out.ap().opt()])
            ar_w2 = nc.gpsimd.collective_compute(
                "AllReduce", mybir.AluOpType.add, replica_groups=RG,
                ins=[warm_out.ap().opt()], outs=[warm_out2.ap().opt()])

            # rotating load pools
            vp_cm = tc.tile_pool(name="vp", bufs=2)
            vp = vp_cm.__enter__()
            vbp_cm = tc.tile_pool(name="vbp", bufs=4)
            vbp = vbp_cm.__enter__()
            kp_cm = tc.tile_pool(name="kp", bufs=2)
            kp = kp_cm.__enter__()

            s_cur = [None] * BL
            s_h = {}
            attn = {}
            vT = {0: [], 1: []}

            def stage_A(b, h):
                """returns (t_pool_cm, t_tile); caller's stage_B closes it"""
                t_cm = tc.tile_pool(name=f"t{b}{h}", bufs=1)
                tp = t_cm.__enter__()
                t_bh = tp.tile([128, M_T, 512], BF16, name=f"t{b}_{h}")
                with (
                    tc.tile_pool(name=f"ktm{b}{h}", bufs=2) as ktp,
                    tc.tile_pool(name=f"A{b}{h}ps", bufs=3, space="PSUM") as aps,
                    tc.tile_pool(name=f"A{b}{h}tp", bufs=2, space="PSUM") as tps,
                ):
                    for m in range(M_T):
                        ktm = ktp.tile([128, KC, 128], BF16, tag="ktm",
                                       name=f"ktm{b}_{h}_{m}")
                        if h == 0:
                            kslab = kp.tile([128, D], F32, tag="kslab",
                                            name=f"kslab{b}_{m}")
                            nc.sync.dma_start(
                                kslab[:], k2.ap()[b, m * 128:(m + 1) * 128, :])
                            for g in range(2):
                                ptr = tps.tile([128, 512], F32, tag="ptr",
                                               name=f"ptr{b}_{m}_{g}")
                                for i in range(4):
                                    kc = g * 4 + i
                                    nc.tensor.transpose(
                                        ptr[:, i * 128:(i + 1) * 128],
                                        kslab[:, kc * 128:(kc + 1) * 128],
                                        ident[:])
                                nc.vector.tensor_copy(
                                    ktm[:, g * 4:(g + 1) * 4, :], ptr[:])
                            nc.sync.dma_start(kt_d.ap()[b][:, m, :, :], ktm[:])
                        else:
                            nc.sync.dma_start(ktm[:], kt_d.ap()[b][:, m, :, :])
                        ps = aps.tile([128, 512], F32, tag="aps",
                                      name=f"aps{b}_{h}_{m}")
                        for kc in range(KC):
                            nc.tensor.matmul(
                                ps[:], ktm[:, kc, :], W_h[h][:, kc, :],
                                start=(kc == 0), stop=(kc == KC - 1))
                        nc.vector.tensor_add(ps[:], ps[:], fb[(b, h)][:])
                        nc.scalar.activation(t_bh[:, m, :], ps[:], AF.Tanh)
                return t_cm, t_bh

            def stage_B(b, h, t_cm, t_bh):
                s_t = spool.tile([128, KC, 512], F32, tag=f"s{b}",
                                 name=f"s{b}_{h}")
                s_cur[b] = s_t
                with tc.tile_pool(name=f"B{b}{h}ps", bufs=1,
                                  space="PSUM") as bps:
                    psb = [bps.tile([128, 512], F32, tag=f"pb{dt}",
                                    name=f"pb{b}_{h}_{dt}") for dt in range(KC)]
                    for m in range(M_T):
                        vf = vp.tile([128, D], F32, tag="vf",
                                     name=f"vf{b}_{h}_{m}")
                        if h == 0:
                            nc.scalar.dma_start(
                                vf[:], v2.ap()[b, m * 128:(m + 1) * 128, :])
                        else:
                            nc.sync.dma_start(
                                vf[:], v2.ap()[b, m * 128:(m + 1) * 128, :])
                        vb = vbp.tile([128, D], BF16, tag="vb",
                                      name=f"vbb{b}_{h}_{m}")
                        nc.vector.tensor_copy(vb[:], vf[:])
                        if h == 1:
                            vt = vtp.tile([128, KC, 128], BF16,
                                          tag=f"vt{b}_{m}", name=f"vt{b}_{m}")
                            nc.scalar.dma_start(vt[:], vb[:], transpose=True)
                            vT[b].append(vt)
                        for dt in range(KC):
                            nc.tensor.matmul(
                                psb[dt][:],
                                vb[:, dt * 128:(dt + 1) * 128],
                                t_bh[:, m, :],
                                start=(m == 0), stop=(m == M_T - 1))
                    for dt in range(KC):
                        nc.vector.tensor_copy(s_t[:, dt, :], psb[dt][:])
                t_cm.__exit__(None, None, None)

            prev_ar = [ar_w2]

            def sm_max(h):
                s_h[h] = list(s_cur)
                for b in range(BL):
                    attn[(b, h)] = attnp.tile([128, KC, 512], BF16,
                                              tag=f"at{b}{h}",
                                              name=f"attn{b}_{h}")
                for c in range(ARC):
                    dsl = slice(2 * c, 2 * c + 2)
                    mx = smp.tile([128, 2, 512], BF16, tag="bc1",
                                  name=f"mx{h}_{c}")
                    nc.vector.tensor_max(mx[:], s_h[h][0][:, dsl, :],
                                         s_h[h][1][:, dsl, :])
                    nc.scalar.dma_start(mx_in[h].ap()[:, dsl, :], mx[:])
                ar_mx = nc.gpsimd.collective_compute(
                    "AllReduce", mybir.AluOpType.max, replica_groups=RG,
                    ins=[mx_in[h].ap().opt()], outs=[mx_out[h].ap().opt()])
                tile.add_dep_helper(ar_mx.ins, prev_ar[0].ins, sync=False,
                                    reason="serialize collectives")
                prev_ar[0] = ar_mx

            def sm_exp(h, eng):
                for c in range(ARC):
                    dsl = slice(2 * c, 2 * c + 2)
                    gmxb = smp.tile([128, 2, 512], BF16, tag="bc2",
                                    name=f"gmxb{h}_{c}")
                    nc.gpsimd.dma_start(gmxb[:], mx_out[h].ap()[:, dsl, :])
                    for b in range(BL):
                        eng.tensor_sub(s_h[h][b][:, dsl, :],
                                       s_h[h][b][:, dsl, :], gmxb[:])
                        nc.scalar.activation(attn[(b, h)][:, dsl, :],
                                             s_h[h][b][:, dsl, :], AF.Exp)
                    sm = smp.tile([128, 2, 512], BF16, tag="bc1",
                                  name=f"sm{h}_{c}")
                    eng.tensor_add(sm[:], attn[(0, h)][:, dsl, :],
                                   attn[(1, h)][:, dsl, :])
                    nc.scalar.dma_start(sm_in[h].ap()[:, dsl, :], sm[:])
                ar_sm = nc.gpsimd.collective_compute(
                    "AllReduce", mybir.AluOpType.add, replica_groups=RG,
                    ins=[sm_in[h].ap().opt()], outs=[sm_out[h].ap().opt()])
                tile.add_dep_helper(ar_sm.ins, prev_ar[0].ins, sync=False,
                                    reason="serialize collectives")
                prev_ar[0] = ar_sm

            def sm_rec(h):
                # rec = 1/Z via fast approx (Z >= 1, so no edge cases);
                # attn = p * rec in place (bf16)
                for c in range(2 * ARC):
                    dsl = slice(c, c + 1)
                    zz = smp.tile([128, 1, 512], BF16, tag="bc2",
                                  name=f"zz{h}_{c}")
                    nc.gpsimd.dma_start(zz[:], sm_out[h].ap()[:, dsl, :])
                    zf = smp.tile([128, 1, 512], F32, tag="zf",
                                  name=f"zf{h}_{c}")
                    nc.vector.tensor_copy(zf[:], zz[:])
                    rec = smp.tile([128, 1, 512], F32, tag="rec",
                                   name=f"rec{h}_{c}")
                    nc.vector.reciprocal_approx_fast(rec[:], zf[:])
                    for b in range(BL):
                        nc.vector.tensor_mul(attn[(b, h)][:, dsl, :],
                                             attn[(b, h)][:, dsl, :], rec[:])

            # ======== main schedule ========
            t_cm, t_bh = stage_A(0, 0)
            stage_B(0, 0, t_cm, t_bh)
            t_cm, t_bh = stage_A(1, 0)
            load_W_half(1)          # rotate W to the h1 half during A10/B10
            stage_B(1, 0, t_cm, t_bh)

            sm_max(0)
            t_cm, t_bh = stage_A(0, 1)
            sm_exp(0, nc.gpsimd)
            stage_B(0, 1, t_cm, t_bh)
            t_cm, t_bh = stage_A(1, 1)
            stage_B(1, 1, t_cm, t_bh)
            sm_rec(0)

            kp_cm.__exit__(None, None, None)
            vbp_cm.__exit__(None, None, None)
            vp_cm.__exit__(None, None, None)

            sm_max(1)

            # ======== stage C (interleaved with softmax h1 tail) ========
            cps_cm = tc.tile_pool(name="cps", bufs=6, space="PSUM")
            cps = cps_cm.__enter__()

            def stage_c(b, h):
                he = slice(h * 512, (h + 1) * 512)
                for m in range(M_T):
                    ps = cps.tile([128, 512], F32, tag="cps",
                                  name=f"cps{b}_{h}_{m}")
                    for kc in range(KC):
                        nc.tensor.matmul(
                            ps[:], vT[b][m][:, kc, :],
                            attn[(b, h)][:, kc, :],
                            start=(kc == 0), stop=(kc == KC - 1))
                    ost = cpool.tile([128, 512], F32, tag="ost",
                                     name=f"ost{b}_{h}_{m}")
                    nc.scalar.copy(ost[:], ps[:])
                    nc.sync.dma_start(
                        out2.ap()[b, m * 128:(m + 1) * 128, he], ost[:])

            stage_c(0, 0)
            sm_exp(1, nc.vector)
            stage_c(1, 0)
            sm_rec(1)

            wp_cm.__exit__(None, None, None)
            sp_cm.__exit__(None, None, None)
            sm_cm.__exit__(None, None, None)

            stage_c(0, 1)
            stage_c(1, 1)

            cps_cm.__exit__(None, None, None)
            ap_cm.__exit__(None, None, None)
            vt_cm.__exit__(None, None, None)
            cp_cm.__exit__(None, None, None)

    nc.compile()
    return nc


_NC = None


def _get_nc():
    global _NC
    if _NC is None:
        _NC = build()
    return _NC


def kernel(q, k, v, W, U):
    q = np.ascontiguousarray(np.asarray(q, dtype=np.float32))
    k = np.ascontiguousarray(np.asarray(k, dtype=np.float32))
    v = np.ascontiguousarray(np.asarray(v, dtype=np.float32))
    W = np.ascontiguousarray(np.asarray(W, dtype=np.float32))
    U = np.ascontiguousarray(np.asarray(U, dtype=np.float32))

    nc = _get_nc()
    in_maps = [
        {
            "q2": q[c * BL:(c + 1) * BL],
            "k2": k[c * BL:(c + 1) * BL],
            "v2": v[c * BL:(c + 1) * BL],
            "W": W,
            "U": U,
        }
        for c in range(N_CORES)
    ]
    res = run_bass_kernel_spmd(nc, in_maps, core_ids=list(range(N_CORES)))
    out = np.concatenate([res.results[c]["out"] for c in range(N_CORES)], axis=0)
    return out.astype(np.float32)


if __name__ == "__main__":
    rng = np.random.default_rng(0)
    q = rng.standard_normal((B, D), dtype=np.float32)
    k = rng.standard_normal((B, S, D), dtype=np.float32)
    v = rng.standard_normal((B, S, D), dtype=np.float32)
    W = (rng.standard_normal((D, D), dtype=np.float32) / np.sqrt(D)).astype(np.float32)
    U = (rng.standard_normal((D, D), dtype=np.float32) / np.sqrt(D)).astype(np.float32)
    out = kernel(q=q, k=k, v=v, W=W, U=U)
    print("out", out.shape, out.dtype, float(np.abs(out).mean()))


# revision 16
# speedup vs baseline: 1.0867x; 1.0214x over previous
"""Distributed Trainium2 kernel for nn_Attention_31104153157828.

Computation (B=16, S=2048, D=1024):
    fac1 = k @ W                     [B,S,D]
    fac2 = (q @ U)[:, None, :]       [B,1,D]
    t    = tanh(fac1 + fac2)
    s    = einsum('bsd,bse->bde', v, t)      [B,D,D]
    attn = softmax(s, axis=0)                 (softmax over BATCH)
    out  = einsum('bsd,bde->bse', v, attn)   [B,S,D]

Sharding: data-parallel over batch, 2 batches per core on 8 cores.
The batch-axis softmax needs cross-core AllReduce of max and sum(exp)
over the [D,D] logit matrix (per e-half, bf16 payload).

PE issue rate is ~263ns per 512-row matmul regardless of dtype, so the
schedule minimizes PE instructions and keeps every engine queue free of
head-of-line blocking:
  - k is PE-transposed once per batch (h0 pass); kT is round-tripped
    through DRAM for the h1 pass instead of re-transposing.
  - fac2 is added via a DVE broadcast-add into PSUM, not per-tile K=1
    matmuls.
  - softmax is split into phases (max/AR, exp/AR, rec/mul) and emitted
    interleaved with stage C so AR-gated ops never block C's queue work.
  - 1/Z uses cast + reciprocal_approx_fast (Z >= 1 always).

Emission order:
  A00 B00 A10 B10 | mx0 | A01 | exp0 | B01 | rec0 | A11 B11
  | mx1 | C00 | exp1 | C10 | rec1 | C01 C11
"""
import numpy as np
import concourse.bass as bass
import concourse.bacc as bacc
import concourse.tile as tile
import concourse.mybir as mybir
from concourse.bass_utils import run_bass_kernel_spmd

F32 = mybir.dt.float32
BF16 = mybir.dt.bfloat16
AF = mybir.ActivationFunctionType

B, S, D = 16, 2048, 1024
N_CORES = 8
BL = B // N_CORES          # local batches per core = 2
M_T = S // 128             # 16 s-tiles
KC = D // 128              # 8 contraction chunks (d)
EH = 2                     # e halves of 512
ARC = 4                    # AllReduce chunks (pairs of d-tiles)
RG = [list(range(N_CORES))]


def build():
    nc = bacc.Bacc("TRN2", target_bir_lowering=False, debug=False,
                   num_devices=N_CORES)

    q2 = nc.dram_tensor("q2", [BL, D], F32, kind="ExternalInput")
    k2 = nc.dram_tensor("k2", [BL, S, D], F32, kind="ExternalInput")
    v2 = nc.dram_tensor("v2", [BL, S, D], F32, kind="ExternalInput")
    Wd = nc.dram_tensor("W", [D, D], F32, kind="ExternalInput")
    Ud = nc.dram_tensor("U", [D, D], F32, kind="ExternalInput")
    out2 = nc.dram_tensor("out", [BL, S, D], F32, kind="ExternalOutput")

    # kT bounce (bf16), written during h0 A passes, read during h1
    kt_d = nc.dram_tensor("kt_d", [BL, 128, M_T, KC, 128], BF16)

    # collective bounce buffers, one set per e-half
    mx_in = [nc.dram_tensor(f"mx_in{h}", [128, KC, 512], BF16) for h in range(EH)]
    mx_out = [nc.dram_tensor(f"mx_out{h}", [128, KC, 512], BF16) for h in range(EH)]
    sm_in = [nc.dram_tensor(f"sm_in{h}", [128, KC, 512], BF16) for h in range(EH)]
    sm_out = [nc.dram_tensor(f"sm_out{h}", [128, KC, 512], BF16) for h in range(EH)]

    warm_in = nc.dram_tensor("warm_in", [128, 16], F32)
    warm_out = nc.dram_tensor("warm_out", [128, 16], F32)
    warm_out2 = nc.dram_tensor("warm_out2", [128, 16], F32)

    ident_d = nc.inline_tensor(np.eye(128, dtype=np.float32), name="ident")
    ones_d = nc.inline_tensor(np.ones((1, 128), np.float32), name="ones1")

    with tile.TileContext(nc) as tc:
        with tc.tile_pool(name="rp", bufs=1) as rp:
            ident = rp.tile([128, 128], F32, name="ident_t")
            nc.sync.dma_start(ident[:], ident_d.ap())
            ones_bf = rp.tile([1, 128], BF16, name="ones_bf")
            nc.gpsimd.dma_start(ones_bf[:], ones_d.ap())
            wtile = rp.tile([128, 16], F32, name="wtile")
            nc.gpsimd.dma_start(wtile[:], ident_d.ap()[:, 0:16])
            nc.gpsimd.dma_start(warm_in.ap(), wtile[:])

            # long-lived pools first (LIFO release discipline)
            cp_cm = tc.tile_pool(name="cpool", bufs=2)
            cpool = cp_cm.__enter__()
            vt_cm = tc.tile_pool(name="vtp", bufs=1)
            vtp = vt_cm.__enter__()
            ap_cm = tc.tile_pool(name="attnp", bufs=1)
            attnp = ap_cm.__enter__()
            sm_cm = tc.tile_pool(name="smp", bufs=2)
            smp = sm_cm.__enter__()
            sp_cm = tc.tile_pool(name="spool", bufs=1)
            spool = sp_cm.__enter__()
            wp_cm = tc.tile_pool(name="wp", bufs=1)
            wp = wp_cm.__enter__()

            # W bf16 (half at a time, tag-rotated) + fac2 broadcast tiles
            W_h = {}
            fb = {}

            def load_W_half(h):
                W_h[h] = wp.tile([128, KC, 512], BF16, tag="Wh", name=f"W_h{h}")
                with tc.tile_pool(name=f"wtp{h}", bufs=2) as wtp:
                    for kc in range(KC):
                        wtmp = wtp.tile([128, 512], F32, tag="wtmp",
                                        name=f"wt{h}_{kc}")
                        nc.scalar.dma_start(
                            wtmp[:],
                            Wd.ap().rearrange("(kc p) e -> p kc e", p=128)
                            [:, kc, h * 512:(h + 1) * 512])
                        nc.vector.tensor_copy(W_h[h][:, kc, :], wtmp[:])

            # fac2 = q @ U -> broadcast tiles fb[(b,h)] = [128,512] bf16
            # (U first: the fb chain is the longest pole for A00's first tanh)
            with (
                tc.tile_pool(name="f2u", bufs=1) as f2u,
                tc.tile_pool(name="f2", bufs=2) as f2p,
                tc.tile_pool(name="f2ps", bufs=2, space="PSUM") as f2ps,
            ):
                U_bf = f2u.tile([128, KC, D], BF16, name="U_bf")
                with tc.tile_pool(name="utp", bufs=2) as utp:
                    for kc in range(KC):
                        utmp = utp.tile([128, D], F32, tag="utmp",
                                        name=f"ut{kc}")
                        nc.scalar.dma_start(
                            utmp[:],
                            Ud.ap().rearrange("(kc p) e -> p kc e", p=128)[:, kc, :])
                        nc.vector.tensor_copy(U_bf[:, kc, :], utmp[:])
                fac2 = f2u.tile([1, BL, D], BF16, name="fac2")
                for b in range(BL):
                    qcol_f = f2p.tile([128, KC], F32, tag="qcf", name=f"qcf{b}")
                    nc.gpsimd.dma_start(
                        qcol_f[:], q2.ap()[b].rearrange("(kc p) -> p kc", p=128))
                    qcol = f2p.tile([128, KC], BF16, tag="qcb", name=f"qcb{b}")
                    nc.vector.tensor_copy(qcol[:], qcol_f[:])
                    for h in range(EH):
                        ps = f2ps.tile([1, 512], F32, tag="f2ps",
                                       name=f"f2ps{b}_{h}")
                        for kc in range(KC):
                            nc.tensor.matmul(ps[:], qcol[:, kc:kc + 1],
                                             U_bf[:, kc, h * 512:(h + 1) * 512],
                                             start=(kc == 0), stop=(kc == KC - 1))
                        nc.scalar.copy(fac2[0:1, b, h * 512:(h + 1) * 512], ps[:])
                # broadcast fac2 across partitions via K=1 matmul
                for b in range(BL):
                    for h in range(EH):
                        psb = f2ps.tile([128, 512], F32, tag="fbps",
                                        name=f"fbps{b}_{h}")
                        nc.tensor.matmul(psb[:], ones_bf[:],
                                         fac2[0:1, b, h * 512:(h + 1) * 512],
                                         start=True, stop=True)
                        fb[(b, h)] = wp.tile([128, 512], BF16, tag=f"fb{b}{h}",
                                             name=f"fb{b}_{h}")
                        nc.vector.tensor_copy(fb[(b, h)][:], psb[:])

            load_W_half(0)

            # warm up the collective machinery (after the setup DMAs so the
            # gpsimd queue isn't blocked while the barrier settles)
            ar_w1 = nc.gpsimd.collective_compute(
                "AllReduce", mybir.AluOpType.max, replica_groups=RG,
                ins=[warm_in.ap().opt()], outs=[warm_out.ap().opt()])
            ar_w2 = nc.gpsimd.collective_compute(
                "AllReduce", mybir.AluOpType.add, replica_groups=RG,
                ins=[warm_out.ap().opt()], outs=[warm_out2.ap().opt()])

            # rotating load pools
            vp_cm = tc.tile_pool(name="vp", bufs=2)
            vp = vp_cm.__enter__()
            vbp_cm = tc.tile_pool(name="vbp", bufs=4)
            vbp = vbp_cm.__enter__()
            kp_cm = tc.tile_pool(name="kp", bufs=2)
            kp = kp_cm.__enter__()

            s_cur = [None] * BL
            s_h = {}
            attn = {}
            vT = {0: [], 1: []}

            def stage_A(b, h):
                """returns (t_pool_cm, t_tile); caller's stage_B closes it"""
                t_cm = tc.tile_pool(name=f"t{b}{h}", bufs=1)
                tp = t_cm.__enter__()
                t_bh = tp.tile([128, M_T, 512], BF16, name=f"t{b}_{h}")
                with (
                    tc.tile_pool(name=f"ktm{b}{h}", bufs=2) as ktp,
                    tc.tile_pool(name=f"A{b}{h}ps", bufs=3, space="PSUM") as aps,
                    tc.tile_pool(name=f"A{b}{h}tp", bufs=2, space="PSUM") as tps,
                ):
                    for m in range(M_T):
                        ktm = ktp.tile([128, KC, 128], BF16, tag="ktm",
                                       name=f"ktm{b}_{h}_{m}")
                        if h == 0:
                            kslab = kp.tile([128, D], F32, tag="kslab",
                                            name=f"kslab{b}_{m}")
                            nc.sync.dma_start(
                                kslab[:], k2.ap()[b, m * 128:(m + 1) * 128, :])
                            for g in range(2):
                                ptr = tps.tile([128, 512], F32, tag="ptr",
                                               name=f"ptr{b}_{m}_{g}")
                                for i in range(4):
                                    kc = g * 4 + i
                                    nc.tensor.transpose(
                                        ptr[:, i * 128:(i + 1) * 128],
                                        kslab[:, kc * 128:(kc + 1) * 128],
                                        ident[:])
                                nc.vector.tensor_copy(
                                    ktm[:, g * 4:(g + 1) * 4, :], ptr[:])
                            nc.sync.dma_start(kt_d.ap()[b][:, m, :, :], ktm[:])
                        else:
                            nc.sync.dma_start(ktm[:], kt_d.ap()[b][:, m, :, :])
                        ps = aps.tile([128, 512], F32, tag="aps",
                                      name=f"aps{b}_{h}_{m}")
                        for kc in range(KC):
                            nc.tensor.matmul(
                                ps[:], ktm[:, kc, :], W_h[h][:, kc, :],
                                start=(kc == 0), stop=(kc == KC - 1))
                        nc.vector.tensor_add(ps[:], ps[:], fb[(b, h)][:])
                        nc.scalar.activation(t_bh[:, m, :], ps[:], AF.Tanh)
                return t_cm, t_bh

            def stage_B(b, h, t_cm, t_bh):
                s_t = spool.tile([128, KC, 512], F32, tag=f"s{b}",
                                 name=f"s{b}_{h}")
                s_cur[b] = s_t
                with tc.tile_pool(name=f"B{b}{h}ps", bufs=1,
                                  space="PSUM") as bps:
                    psb = [bps.tile([128, 512], F32, tag=f"pb{dt}",
                                    name=f"pb{b}_{h}_{dt}") for dt in range(KC)]
                    for m in range(M_T):
                        vf = vp.tile([128, D], F32, tag="vf",
                                     name=f"vf{b}_{h}_{m}")
                        nc.scalar.dma_start(
                            vf[:], v2.ap()[b, m * 128:(m + 1) * 128, :])
                        vb = vbp.tile([128, D], BF16, tag="vb",
                                      name=f"vbb{b}_{h}_{m}")
                        nc.vector.tensor_copy(vb[:], vf[:])
                        if h == 1:
                            vt = vtp.tile([128, KC, 128], BF16,
                                          tag=f"vt{b}_{m}", name=f"vt{b}_{m}")
                            nc.scalar.dma_start(vt[:], vb[:], transpose=True)
                            vT[b].append(vt)
                        for dt in range(KC):
                            nc.tensor.matmul(
                                psb[dt][:],
                                vb[:, dt * 128:(dt + 1) * 128],
                                t_bh[:, m, :],
                                start=(m == 0), stop=(m == M_T - 1))
                    for dt in range(KC):
                        nc.vector.tensor_copy(s_t[:, dt, :], psb[dt][:])
                t_cm.__exit__(None, None, None)

            prev_ar = [ar_w2]

            def sm_max(h):
                s_h[h] = list(s_cur)
                for b in range(BL):
                    attn[(b, h)] = attnp.tile([128, KC, 512], BF16,
                                              tag=f"at{b}{h}",
                                              name=f"attn{b}_{h}")
                for c in range(ARC):
                    dsl = slice(2 * c, 2 * c + 2)
                    mx = smp.tile([128, 2, 512], BF16, tag="bc1",
                                  name=f"mx{h}_{c}")
                    nc.vector.tensor_max(mx[:], s_h[h][0][:, dsl, :],
                                         s_h[h][1][:, dsl, :])
                    nc.gpsimd.dma_start(mx_in[h].ap()[:, dsl, :], mx[:])
                ar_mx = nc.gpsimd.collective_compute(
                    "AllReduce", mybir.AluOpType.max, replica_groups=RG,
                    ins=[mx_in[h].ap().opt()], outs=[mx_out[h].ap().opt()])
                tile.add_dep_helper(ar_mx.ins, prev_ar[0].ins, sync=False,
                                    reason="serialize collectives")
                prev_ar[0] = ar_mx

            def sm_exp(h, eng):
                for c in range(ARC):
                    dsl = slice(2 * c, 2 * c + 2)
                    gmxb = smp.tile([128, 2, 512], BF16, tag="bc2",
                                    name=f"gmxb{h}_{c}")
                    nc.gpsimd.dma_start(gmxb[:], mx_out[h].ap()[:, dsl, :])
                    for b in range(BL):
                        eng.tensor_sub(s_h[h][b][:, dsl, :],
                                       s_h[h][b][:, dsl, :], gmxb[:])
                        nc.scalar.activation(attn[(b, h)][:, dsl, :],
                                             s_h[h][b][:, dsl, :], AF.Exp)
                    sm = smp.tile([128, 2, 512], BF16, tag="bc1",
                                  name=f"sm{h}_{c}")
                    eng.tensor_add(sm[:], attn[(0, h)][:, dsl, :],
                                   attn[(1, h)][:, dsl, :])
                    nc.gpsimd.dma_start(sm_in[h].ap()[:, dsl, :], sm[:])
                ar_sm = nc.gpsimd.collective_compute(
                    "AllReduce", mybir.AluOpType.add, replica_groups=RG,
                    ins=[sm_in[h].ap().opt()], outs=[sm_out[h].ap().opt()])
                tile.add_dep_helper(ar_sm.ins, prev_ar[0].ins, sync=False,
                                    reason="serialize collectives")
                prev_ar[0] = ar_sm

            def sm_rec(h):
                # rec = 1/Z via fast approx (Z >= 1, so no edge cases);
                # attn = p * rec in place (bf16)
                for c in range(2 * ARC):
                    dsl = slice(c, c + 1)
                    zz = smp.tile([128, 1, 512], BF16, tag="bc2",
                                  name=f"zz{h}_{c}")
                    nc.gpsimd.dma_start(zz[:], sm_out[h].ap()[:, dsl, :])
                    zf = smp.tile([128, 1, 512], F32, tag="zf",
                                  name=f"zf{h}_{c}")
                    nc.vector.tensor_copy(zf[:], zz[:])
                    rec = smp.tile([128, 1, 512], F32, tag="rec",
                                   name=f"rec{h}_{c}")
                    nc.vector.reciprocal_approx_fast(rec[:], zf[:])
                    for b in range(BL):
                        nc.vector.tensor_mul(attn[(b, h)][:, dsl, :],
                                             attn[(b, h)][:, dsl, :], rec[:])

            # ======== main schedule ========
            t_cm, t_bh = stage_A(0, 0)
            stage_B(0, 0, t_cm, t_bh)
            t_cm, t_bh = stage_A(1, 0)
            load_W_half(1)          # rotate W to the h1 half during A10/B10
            stage_B(1, 0, t_cm, t_bh)

            sm_max(0)
            t_cm, t_bh = stage_A(0, 1)
            sm_exp(0, nc.gpsimd)
            stage_B(0, 1, t_cm, t_bh)
            t_cm, t_bh = stage_A(1, 1)
            stage_B(1, 1, t_cm, t_bh)
            sm_rec(0)

            kp_cm.__exit__(None, None, None)
            vbp_cm.__exit__(None, None, None)
            vp_cm.__exit__(None, None, None)

            sm_max(1)

            # ======== stage C (interleaved with softmax h1 tail) ========
            cps_cm = tc.tile_pool(name="cps", bufs=6, space="PSUM")
            cps = cps_cm.__enter__()

            def stage_c(b, h):
                he = slice(h * 512, (h + 1) * 512)
                for m in range(M_T):
                    ps = cps.tile([128, 512], F32, tag="cps",
                                  name=f"cps{b}_{h}_{m}")
                    for kc in range(KC):
                        nc.tensor.matmul(
                            ps[:], vT[b][m][:, kc, :],
                            attn[(b, h)][:, kc, :],
                            start=(kc == 0), stop=(kc == KC - 1))
                    ost = cpool.tile([128, 512], F32, tag="ost",
                                     name=f"ost{b}_{h}_{m}")
                    nc.scalar.copy(ost[:], ps[:])
                    nc.sync.dma_start(
                        out2.ap()[b, m * 128:(m + 1) * 128, he], ost[:])

            stage_c(0, 0)
            sm_exp(1, nc.vector)
            stage_c(1, 0)
            sm_rec(1)

            wp_cm.__exit__(None, None, None)
            sp_cm.__exit__(None, None, None)
            sm_cm.__exit__(None, None, None)

            stage_c(0, 1)
            stage_c(1, 1)

            cps_cm.__exit__(None, None, None)
            ap_cm.__exit__(None, None, None)
            vt_cm.__exit__(None, None, None)
            cp_cm.__exit__(None, None, None)

    nc.compile()
    return nc


_NC = None


def _get_nc():
    global _NC
    if _NC is None:
        _NC = build()
    return _NC


def kernel(q, k, v, W, U):
    q = np.ascontiguousarray(np.asarray(q, dtype=np.float32))
    k = np.ascontiguousarray(np.asarray(k, dtype=np.float32))
    v = np.ascontiguousarray(np.asarray(v, dtype=np.float32))
    W = np.ascontiguousarray(np.asarray(W, dtype=np.float32))
    U = np.ascontiguousarray(np.asarray(U, dtype=np.float32))

    nc = _get_nc()
    in_maps = [
        {
            "q2": q[c * BL:(c + 1) * BL],
            "k2": k[c * BL:(c + 1) * BL],
            "v2": v[c * BL:(c + 1) * BL],
            "W": W,
            "U": U,
        }
        for c in range(N_CORES)
    ]
    res = run_bass_kernel_spmd(nc, in_maps, core_ids=list(range(N_CORES)))
    out = np.concatenate([res.results[c]["out"] for c in range(N_CORES)], axis=0)
    return out.astype(np.float32)


if __name__ == "__main__":
    rng = np.random.default_rng(0)
    q = rng.standard_normal((B, D), dtype=np.float32)
    k = rng.standard_normal((B, S, D), dtype=np.float32)
    v = rng.standard_normal((B, S, D), dtype=np.float32)
    W = (rng.standard_normal((D, D), dtype=np.float32) / np.sqrt(D)).astype(np.float32)
    U = (rng.standard_normal((D, D), dtype=np.float32) / np.sqrt(D)).astype(np.float32)
    out = kernel(q=q, k=k, v=v, W=W, U=U)
    print("out", out.shape, out.dtype, float(np.abs(out).mean()))


# revision 19
# speedup vs baseline: 1.1095x; 1.0209x over previous
"""Distributed Trainium2 kernel for nn_Attention_31104153157828.

Computation (B=16, S=2048, D=1024):
    fac1 = k @ W                     [B,S,D]
    fac2 = (q @ U)[:, None, :]       [B,1,D]
    t    = tanh(fac1 + fac2)
    s    = einsum('bsd,bse->bde', v, t)      [B,D,D]
    attn = softmax(s, axis=0)                 (softmax over BATCH)
    out  = einsum('bsd,bde->bse', v, attn)   [B,S,D]

Sharding: data-parallel over batch, 2 batches per core on 8 cores.
The batch-axis softmax needs cross-core AllReduce of max and sum(exp)
over the [D,D] logit matrix (per e-half, bf16 payload).

PE issue rate is ~263ns per 512-row matmul regardless of dtype, so the
schedule minimizes PE instructions and keeps every engine queue free of
head-of-line blocking:
  - k is PE-transposed once per batch (h0 pass); kT is round-tripped
    through DRAM for the h1 pass instead of re-transposing.
  - fac2 is added via a DVE broadcast-add into PSUM, not per-tile K=1
    matmuls.
  - softmax is split into phases (max/AR, exp/AR, rec/mul) and emitted
    interleaved with stage C so AR-gated ops never block C's queue work.
  - 1/Z uses cast + reciprocal_approx_fast (Z >= 1 always).

Emission order:
  A00 B00 A10 B10 | mx0 | A01 | exp0 | B01 | rec0 | A11 B11
  | mx1 | C00 | exp1 | C10 | rec1 | C01 C11
"""
import numpy as np
import concourse.bass as bass
import concourse.bacc as bacc
import concourse.tile as tile
import concourse.mybir as mybir
from concourse.bass_utils import run_bass_kernel_spmd

F32 = mybir.dt.float32
BF16 = mybir.dt.bfloat16
AF = mybir.ActivationFunctionType

B, S, D = 16, 2048, 1024
N_CORES = 8
BL = B // N_CORES          # local batches per core = 2
M_T = S // 128             # 16 s-tiles
KC = D // 128              # 8 contraction chunks (d)
EH = 2                     # e halves of 512
ARC = 4                    # AllReduce chunks (pairs of d-tiles)
RG = [list(range(N_CORES))]


def build():
    nc = bacc.Bacc("TRN2", target_bir_lowering=False, debug=False,
                   num_devices=N_CORES)

    q2 = nc.dram_tensor("q2", [BL, D], F32, kind="ExternalInput")
    k2 = nc.dram_tensor("k2", [BL, S, D], F32, kind="ExternalInput")
    v2 = nc.dram_tensor("v2", [BL, S, D], F32, kind="ExternalInput")
    Wd = nc.dram_tensor("W", [D, D], F32, kind="ExternalInput")
    Ud = nc.dram_tensor("U", [D, D], F32, kind="ExternalInput")
    out2 = nc.dram_tensor("out", [BL, S, D], F32, kind="ExternalOutput")

    # kT bounce (bf16), written during h0 A passes, read during h1
    kt_d = nc.dram_tensor("kt_d", [BL, 128, M_T, KC, 128], BF16)

    # collective bounce buffers, one set per e-half
    mx_in = [nc.dram_tensor(f"mx_in{h}", [128, KC, 512], BF16) for h in range(EH)]
    mx_out = [nc.dram_tensor(f"mx_out{h}", [128, KC, 512], BF16) for h in range(EH)]
    sm_in = [nc.dram_tensor(f"sm_in{h}", [128, KC, 512], BF16) for h in range(EH)]
    sm_out = [nc.dram_tensor(f"sm_out{h}", [128, KC, 512], BF16) for h in range(EH)]

    warm_in = nc.dram_tensor("warm_in", [128, 16], F32)
    warm_out = nc.dram_tensor("warm_out", [128, 16], F32)
    warm_out2 = nc.dram_tensor("warm_out2", [128, 16], F32)

    ident_d = nc.inline_tensor(np.eye(128, dtype=np.float32), name="ident")
    ones_d = nc.inline_tensor(np.ones((1, 128), np.float32), name="ones1")

    with tile.TileContext(nc) as tc:
        with tc.tile_pool(name="rp", bufs=1) as rp:
            ident = rp.tile([128, 128], F32, name="ident_t")
            nc.sync.dma_start(ident[:], ident_d.ap())
            ones_bf = rp.tile([1, 128], BF16, name="ones_bf")
            nc.gpsimd.dma_start(ones_bf[:], ones_d.ap())
            wtile = rp.tile([128, 16], F32, name="wtile")
            nc.gpsimd.dma_start(wtile[:], ident_d.ap()[:, 0:16])
            nc.gpsimd.dma_start(warm_in.ap(), wtile[:])

            # long-lived pools first (LIFO release discipline)
            cp_cm = tc.tile_pool(name="cpool", bufs=2)
            cpool = cp_cm.__enter__()
            vt_cm = tc.tile_pool(name="vtp", bufs=1)
            vtp = vt_cm.__enter__()
            ap_cm = tc.tile_pool(name="attnp", bufs=1)
            attnp = ap_cm.__enter__()
            sm_cm = tc.tile_pool(name="smp", bufs=2)
            smp = sm_cm.__enter__()
            sp_cm = tc.tile_pool(name="spool", bufs=1)
            spool = sp_cm.__enter__()
            wp_cm = tc.tile_pool(name="wp", bufs=1)
            wp = wp_cm.__enter__()

            # W bf16 (half at a time, tag-rotated) + fac2 broadcast tiles
            W_h = {}
            fb = {}

            def load_W_half(h):
                W_h[h] = wp.tile([128, KC, 512], BF16, tag="Wh", name=f"W_h{h}")
                with tc.tile_pool(name=f"wtp{h}", bufs=2) as wtp:
                    for kc in range(KC):
                        wtmp = wtp.tile([128, 512], F32, tag="wtmp",
                                        name=f"wt{h}_{kc}")
                        nc.scalar.dma_start(
                            wtmp[:],
                            Wd.ap().rearrange("(kc p) e -> p kc e", p=128)
                            [:, kc, h * 512:(h + 1) * 512])
                        nc.vector.tensor_copy(W_h[h][:, kc, :], wtmp[:])

            # fac2 = q @ U -> broadcast tiles fb[(b,h)] = [128,512] bf16
            # (U first: the fb chain is the longest pole for A00's first tanh)
            with (
                tc.tile_pool(name="f2u", bufs=1) as f2u,
                tc.tile_pool(name="f2", bufs=2) as f2p,
                tc.tile_pool(name="f2ps", bufs=2, space="PSUM") as f2ps,
            ):
                U_bf = f2u.tile([128, KC, D], BF16, name="U_bf")
                with tc.tile_pool(name="utp", bufs=2) as utp:
                    for kc in range(KC):
                        utmp = utp.tile([128, D], F32, tag="utmp",
                                        name=f"ut{kc}")
                        nc.scalar.dma_start(
                            utmp[:],
                            Ud.ap().rearrange("(kc p) e -> p kc e", p=128)[:, kc, :])
                        nc.vector.tensor_copy(U_bf[:, kc, :], utmp[:])
                fac2 = f2u.tile([1, BL, D], BF16, name="fac2")
                for b in range(BL):
                    qcol_f = f2p.tile([128, KC], F32, tag="qcf", name=f"qcf{b}")
                    nc.gpsimd.dma_start(
                        qcol_f[:], q2.ap()[b].rearrange("(kc p) -> p kc", p=128))
                    qcol = f2p.tile([128, KC], BF16, tag="qcb", name=f"qcb{b}")
                    nc.vector.tensor_copy(qcol[:], qcol_f[:])
                    for h in range(EH):
                        ps = f2ps.tile([1, 512], F32, tag="f2ps",
                                       name=f"f2ps{b}_{h}")
                        for kc in range(KC):
                            nc.tensor.matmul(ps[:], qcol[:, kc:kc + 1],
                                             U_bf[:, kc, h * 512:(h + 1) * 512],
                                             start=(kc == 0), stop=(kc == KC - 1))
                        nc.scalar.copy(fac2[0:1, b, h * 512:(h + 1) * 512], ps[:])
                # broadcast fac2 across partitions via K=1 matmul
                for b in range(BL):
                    for h in range(EH):
                        psb = f2ps.tile([128, 512], F32, tag="fbps",
                                        name=f"fbps{b}_{h}")
                        nc.tensor.matmul(psb[:], ones_bf[:],
                                         fac2[0:1, b, h * 512:(h + 1) * 512],
                                         start=True, stop=True)
                        fb[(b, h)] = wp.tile([128, 512], BF16, tag=f"fb{b}{h}",
                                             name=f"fb{b}_{h}")
                        nc.vector.tensor_copy(fb[(b, h)][:], psb[:])

            load_W_half(0)

            # warm up the collective machinery (after the setup DMAs so the
            # gpsimd queue isn't blocked while the barrier settles)
            ar_w1 = nc.gpsimd.collective_compute(
                "AllReduce", mybir.AluOpType.max, replica_groups=RG,
                ins=[warm_in.ap().opt()], outs=[warm_out.ap().opt()])
            ar_w2 = nc.gpsimd.collective_compute(
                "AllReduce", mybir.AluOpType.add, replica_groups=RG,
                ins=[warm_out.ap().opt()], outs=[warm_out2.ap().opt()])

            # rotating load pools
            vp_cm = tc.tile_pool(name="vp", bufs=2)
            vp = vp_cm.__enter__()
            vbp_cm = tc.tile_pool(name="vbp", bufs=3)
            vbp = vbp_cm.__enter__()
            kp_cm = tc.tile_pool(name="kp", bufs=2)
            kp = kp_cm.__enter__()

            s_cur = [None] * BL
            s_h = {}
            attn = {}
            vT = {0: [], 1: []}

            def stage_A(b, h):
                """returns (t_pool_cm, t_tile); caller's stage_B closes it"""
                t_cm = tc.tile_pool(name=f"t{b}{h}", bufs=1)
                tp = t_cm.__enter__()
                t_bh = tp.tile([128, M_T, 512], BF16, name=f"t{b}_{h}")
                with (
                    tc.tile_pool(name=f"ktm{b}{h}", bufs=2) as ktp,
                    tc.tile_pool(name=f"A{b}{h}ps", bufs=3, space="PSUM") as aps,
                    tc.tile_pool(name=f"A{b}{h}tp", bufs=2, space="PSUM") as tps,
                ):
                    for m in range(M_T):
                        ktm = ktp.tile([128, KC, 128], BF16, tag="ktm",
                                       name=f"ktm{b}_{h}_{m}")
                        if h == 0:
                            kslab = kp.tile([128, D], F32, tag="kslab",
                                            name=f"kslab{b}_{m}")
                            nc.sync.dma_start(
                                kslab[:], k2.ap()[b, m * 128:(m + 1) * 128, :])
                            for g in range(2):
                                ptr = tps.tile([128, 512], F32, tag="ptr",
                                               name=f"ptr{b}_{m}_{g}")
                                for i in range(4):
                                    kc = g * 4 + i
                                    nc.tensor.transpose(
                                        ptr[:, i * 128:(i + 1) * 128],
                                        kslab[:, kc * 128:(kc + 1) * 128],
                                        ident[:])
                                nc.vector.tensor_copy(
                                    ktm[:, g * 4:(g + 1) * 4, :], ptr[:])
                            nc.sync.dma_start(kt_d.ap()[b][:, m, :, :], ktm[:])
                        else:
                            nc.sync.dma_start(ktm[:], kt_d.ap()[b][:, m, :, :])
                        ps = aps.tile([128, 512], F32, tag="aps",
                                      name=f"aps{b}_{h}_{m}")
                        for kc in range(KC):
                            nc.tensor.matmul(
                                ps[:], ktm[:, kc, :], W_h[h][:, kc, :],
                                start=(kc == 0), stop=(kc == KC - 1))
                        nc.vector.tensor_add(ps[:], ps[:], fb[(b, h)][:])
                        nc.scalar.activation(t_bh[:, m, :], ps[:], AF.Tanh)
                return t_cm, t_bh

            def stage_B(b, h, t_cm, t_bh):
                s_t = spool.tile([128, KC, 512], F32, tag=f"s{b}",
                                 name=f"s{b}_{h}")
                s_cur[b] = s_t
                with tc.tile_pool(name=f"B{b}{h}ps", bufs=1,
                                  space="PSUM") as bps:
                    psb = [bps.tile([128, 512], F32, tag=f"pb{dt}",
                                    name=f"pb{b}_{h}_{dt}") for dt in range(KC)]
                    for m in range(M_T):
                        vf = vp.tile([128, D], F32, tag="vf",
                                     name=f"vf{b}_{h}_{m}")
                        if h == 0:
                            nc.scalar.dma_start(
                                vf[:], v2.ap()[b, m * 128:(m + 1) * 128, :])
                        else:
                            nc.sync.dma_start(
                                vf[:], v2.ap()[b, m * 128:(m + 1) * 128, :])
                        vb = vbp.tile([128, D], BF16, tag="vb",
                                      name=f"vbb{b}_{h}_{m}")
                        nc.vector.tensor_copy(vb[:], vf[:])
                        if h == 1:
                            vt = vtp.tile([128, KC, 128], BF16,
                                          tag=f"vt{b}_{m}", name=f"vt{b}_{m}")
                            nc.sync.dma_start(vt[:], vb[:], transpose=True)
                            vT[b].append(vt)
                        for dt in range(KC):
                            nc.tensor.matmul(
                                psb[dt][:],
                                vb[:, dt * 128:(dt + 1) * 128],
                                t_bh[:, m, :],
                                start=(m == 0), stop=(m == M_T - 1))
                    for dt in range(KC):
                        nc.vector.tensor_copy(s_t[:, dt, :], psb[dt][:])
                t_cm.__exit__(None, None, None)

            prev_ar = [ar_w2]

            def sm_max(h):
                s_h[h] = list(s_cur)
                for b in range(BL):
                    attn[(b, h)] = attnp.tile([128, KC, 512], BF16,
                                              tag=f"at{b}{h}",
                                              name=f"attn{b}_{h}")
                for c in range(2 * ARC):
                    dsl = slice(c, c + 1)
                    mx = smp.tile([128, 1, 512], BF16, tag="bc1",
                                  name=f"mx{h}_{c}")
                    nc.vector.tensor_max(mx[:], s_h[h][0][:, dsl, :],
                                         s_h[h][1][:, dsl, :])
                    nc.gpsimd.dma_start(mx_in[h].ap()[:, dsl, :], mx[:])
                ar_mx = nc.gpsimd.collective_compute(
                    "AllReduce", mybir.AluOpType.max, replica_groups=RG,
                    ins=[mx_in[h].ap().opt()], outs=[mx_out[h].ap().opt()])
                tile.add_dep_helper(ar_mx.ins, prev_ar[0].ins, sync=False,
                                    reason="serialize collectives")
                prev_ar[0] = ar_mx

            def sm_exp(h, eng):
                for c in range(2 * ARC):
                    dsl = slice(c, c + 1)
                    gmxb = smp.tile([128, 1, 512], BF16, tag="bc2",
                                    name=f"gmxb{h}_{c}")
                    nc.gpsimd.dma_start(gmxb[:], mx_out[h].ap()[:, dsl, :])
                    for b in range(BL):
                        eng.tensor_sub(s_h[h][b][:, dsl, :],
                                       s_h[h][b][:, dsl, :], gmxb[:])
                        nc.scalar.activation(attn[(b, h)][:, dsl, :],
                                             s_h[h][b][:, dsl, :], AF.Exp)
                    sm = smp.tile([128, 1, 512], BF16, tag="bc1",
                                  name=f"sm{h}_{c}")
                    eng.tensor_add(sm[:], attn[(0, h)][:, dsl, :],
                                   attn[(1, h)][:, dsl, :])
                    nc.gpsimd.dma_start(sm_in[h].ap()[:, dsl, :], sm[:])
                ar_sm = nc.gpsimd.collective_compute(
                    "AllReduce", mybir.AluOpType.add, replica_groups=RG,
                    ins=[sm_in[h].ap().opt()], outs=[sm_out[h].ap().opt()])
                tile.add_dep_helper(ar_sm.ins, prev_ar[0].ins, sync=False,
                                    reason="serialize collectives")
                prev_ar[0] = ar_sm

            def sm_rec(h):
                # rec = 1/Z via fast approx (Z >= 1, so no edge cases);
                # attn = p * rec in place (bf16)
                for c in range(2 * ARC):
                    dsl = slice(c, c + 1)
                    zz = smp.tile([128, 1, 512], BF16, tag="bc2",
                                  name=f"zz{h}_{c}")
                    nc.gpsimd.dma_start(zz[:], sm_out[h].ap()[:, dsl, :])
                    zf = smp.tile([128, 1, 512], F32, tag="zf",
                                  name=f"zf{h}_{c}")
                    nc.vector.tensor_copy(zf[:], zz[:])
                    rec = smp.tile([128, 1, 512], F32, tag="rec",
                                   name=f"rec{h}_{c}")
                    nc.vector.reciprocal_approx_fast(rec[:], zf[:])
                    for b in range(BL):
                        nc.vector.tensor_mul(attn[(b, h)][:, dsl, :],
                                             attn[(b, h)][:, dsl, :], rec[:])

            # ======== main schedule ========
            t_cm, t_bh = stage_A(0, 0)
            stage_B(0, 0, t_cm, t_bh)
            t_cm, t_bh = stage_A(1, 0)
            load_W_half(1)          # rotate W to the h1 half during A10/B10
            stage_B(1, 0, t_cm, t_bh)
            kp_cm.__exit__(None, None, None)

            sm_max(0)
            t_cm0, t_bh0 = stage_A(0, 1)
            t_cm1, t_bh1 = stage_A(1, 1)
            sm_exp(0, nc.gpsimd)
            stage_B(1, 1, t_cm1, t_bh1)
            stage_B(0, 1, t_cm0, t_bh0)
            sm_rec(0)

            vbp_cm.__exit__(None, None, None)
            vp_cm.__exit__(None, None, None)

            sm_max(1)

            # ======== stage C (interleaved with softmax h1 tail) ========
            cps_cm = tc.tile_pool(name="cps", bufs=6, space="PSUM")
            cps = cps_cm.__enter__()

            def stage_c(b, h):
                he = slice(h * 512, (h + 1) * 512)
                for m in range(M_T):
                    ps = cps.tile([128, 512], F32, tag="cps",
                                  name=f"cps{b}_{h}_{m}")
                    for kc in range(KC):
                        nc.tensor.matmul(
                            ps[:], vT[b][m][:, kc, :],
                            attn[(b, h)][:, kc, :],
                            start=(kc == 0), stop=(kc == KC - 1))
                    ost = cpool.tile([128, 512], F32, tag="ost",
                                     name=f"ost{b}_{h}_{m}")
                    nc.scalar.copy(ost[:], ps[:])
                    nc.sync.dma_start(
                        out2.ap()[b, m * 128:(m + 1) * 128, he], ost[:])

            stage_c(0, 0)
            sm_exp(1, nc.vector)
            stage_c(1, 0)
            sm_rec(1)

            wp_cm.__exit__(None, None, None)
            sp_cm.__exit__(None, None, None)
            sm_cm.__exit__(None, None, None)

            stage_c(0, 1)
            stage_c(1, 1)

            cps_cm.__exit__(None, None, None)
            ap_cm.__exit__(None, None, None)
            vt_cm.__exit__(None, None, None)
            cp_cm.__exit__(None, None, None)

    nc.compile()
    return nc


_NC = None


def _get_nc():
    global _NC
    if _NC is None:
        _NC = build()
    return _NC


def kernel(q, k, v, W, U):
    q = np.ascontiguousarray(np.asarray(q, dtype=np.float32))
    k = np.ascontiguousarray(np.asarray(k, dtype=np.float32))
    v = np.ascontiguousarray(np.asarray(v, dtype=np.float32))
    W = np.ascontiguousarray(np.asarray(W, dtype=np.float32))
    U = np.ascontiguousarray(np.asarray(U, dtype=np.float32))

    nc = _get_nc()
    in_maps = [
        {
            "q2": q[c * BL:(c + 1) * BL],
            "k2": k[c * BL:(c + 1) * BL],
            "v2": v[c * BL:(c + 1) * BL],
            "W": W,
            "U": U,
        }
        for c in range(N_CORES)
    ]
    res = run_bass_kernel_spmd(nc, in_maps, core_ids=list(range(N_CORES)))
    out = np.concatenate([res.results[c]["out"] for c in range(N_CORES)], axis=0)
    return out.astype(np.float32)


if __name__ == "__main__":
    rng = np.random.default_rng(0)
    q = rng.standard_normal((B, D), dtype=np.float32)
    k = rng.standard_normal((B, S, D), dtype=np.float32)
    v = rng.standard_normal((B, S, D), dtype=np.float32)
    W = (rng.standard_normal((D, D), dtype=np.float32) / np.sqrt(D)).astype(np.float32)
    U = (rng.standard_normal((D, D), dtype=np.float32) / np.sqrt(D)).astype(np.float32)
    out = kernel(q=q, k=k, v=v, W=W, U=U)
    print("out", out.shape, out.dtype, float(np.abs(out).mean()))


# revision 20
# speedup vs baseline: 1.1440x; 1.0311x over previous
"""Distributed Trainium2 kernel for nn_Attention_31104153157828.

Computation (B=16, S=2048, D=1024):
    fac1 = k @ W                     [B,S,D]
    fac2 = (q @ U)[:, None, :]       [B,1,D]
    t    = tanh(fac1 + fac2)
    s    = einsum('bsd,bse->bde', v, t)      [B,D,D]
    attn = softmax(s, axis=0)                 (softmax over BATCH)
    out  = einsum('bsd,bde->bse', v, attn)   [B,S,D]

Sharding: data-parallel over batch, 2 batches per core on 8 cores.
The batch-axis softmax needs cross-core AllReduce of max and sum(exp)
over the [D,D] logit matrix (per e-half, bf16 payload).

PE issue rate is ~263ns per 512-row matmul regardless of dtype, so the
schedule minimizes PE instructions and keeps every engine queue free of
head-of-line blocking:
  - k is PE-transposed once per batch (h0 pass); kT is round-tripped
    through DRAM for the h1 pass instead of re-transposing.
  - fac2 is added via a DVE broadcast-add into PSUM, not per-tile K=1
    matmuls.
  - softmax is split into phases (max/AR, exp/AR, rec/mul) and emitted
    interleaved with stage C so AR-gated ops never block C's queue work.
  - 1/Z uses cast + reciprocal_approx_fast (Z >= 1 always).

Emission order:
  A00 B00 A10 B10 | mx0 | A01 | exp0 | B01 | rec0 | A11 B11
  | mx1 | C00 | exp1 | C10 | rec1 | C01 C11
"""
import numpy as np
import concourse.bass as bass
import concourse.bacc as bacc
import concourse.tile as tile
import concourse.mybir as mybir
from concourse.bass_utils import run_bass_kernel_spmd

F32 = mybir.dt.float32
BF16 = mybir.dt.bfloat16
AF = mybir.ActivationFunctionType

B, S, D = 16, 2048, 1024
N_CORES = 8
BL = B // N_CORES          # local batches per core = 2
M_T = S // 128             # 16 s-tiles
KC = D // 128              # 8 contraction chunks (d)
EH = 2                     # e halves of 512
ARC = 4                    # AllReduce chunks (pairs of d-tiles)
RG = [list(range(N_CORES))]


def build():
    nc = bacc.Bacc("TRN2", target_bir_lowering=False, debug=False,
                   num_devices=N_CORES)

    q2 = nc.dram_tensor("q2", [BL, D], F32, kind="ExternalInput")
    k2 = nc.dram_tensor("k2", [BL, S, D], F32, kind="ExternalInput")
    v2 = nc.dram_tensor("v2", [BL, S, D], F32, kind="ExternalInput")
    Wd = nc.dram_tensor("W", [D, D], F32, kind="ExternalInput")
    Ud = nc.dram_tensor("U", [D, D], F32, kind="ExternalInput")
    out2 = nc.dram_tensor("out", [BL, S, D], F32, kind="ExternalOutput")

    # kT bounce (bf16), written during h0 A passes, read during h1
    kt_d = nc.dram_tensor("kt_d", [BL, 128, M_T, KC, 128], BF16)

    # collective bounce buffers, one set per e-half
    mx_in = [nc.dram_tensor(f"mx_in{h}", [128, KC, 512], BF16) for h in range(EH)]
    mx_out = [nc.dram_tensor(f"mx_out{h}", [128, KC, 512], BF16) for h in range(EH)]
    sm_in = [nc.dram_tensor(f"sm_in{h}", [128, KC, 512], BF16) for h in range(EH)]
    sm_out = [nc.dram_tensor(f"sm_out{h}", [128, KC, 512], BF16) for h in range(EH)]

    warm_in = nc.dram_tensor("warm_in", [128, 16], F32)
    warm_out = nc.dram_tensor("warm_out", [128, 16], F32)
    warm_out2 = nc.dram_tensor("warm_out2", [128, 16], F32)

    ident_d = nc.inline_tensor(np.eye(128, dtype=np.float32), name="ident")
    ones_d = nc.inline_tensor(np.ones((1, 128), np.float32), name="ones1")

    with tile.TileContext(nc) as tc:
        with tc.tile_pool(name="rp", bufs=1) as rp:
            ident = rp.tile([128, 128], F32, name="ident_t")
            nc.sync.dma_start(ident[:], ident_d.ap())
            ones_bf = rp.tile([1, 128], BF16, name="ones_bf")
            nc.gpsimd.dma_start(ones_bf[:], ones_d.ap())
            wtile = rp.tile([128, 16], F32, name="wtile")
            nc.gpsimd.dma_start(wtile[:], ident_d.ap()[:, 0:16])
            nc.gpsimd.dma_start(warm_in.ap(), wtile[:])

            # long-lived pools first (LIFO release discipline)
            cp_cm = tc.tile_pool(name="cpool", bufs=2)
            cpool = cp_cm.__enter__()
            vt_cm = tc.tile_pool(name="vtp", bufs=1)
            vtp = vt_cm.__enter__()
            ap_cm = tc.tile_pool(name="attnp", bufs=1)
            attnp = ap_cm.__enter__()
            sm_cm = tc.tile_pool(name="smp", bufs=2)
            smp = sm_cm.__enter__()
            sp_cm = tc.tile_pool(name="spool", bufs=1)
            spool = sp_cm.__enter__()
            wp_cm = tc.tile_pool(name="wp", bufs=1)
            wp = wp_cm.__enter__()

            # W bf16 (half at a time, tag-rotated) + fac2 broadcast tiles
            W_h = {}
            fb = {}

            def load_W_half(h):
                W_h[h] = wp.tile([128, KC, 512], BF16, tag="Wh", name=f"W_h{h}")
                with tc.tile_pool(name=f"wtp{h}", bufs=2) as wtp:
                    for kc in range(KC):
                        wtmp = wtp.tile([128, 512], F32, tag="wtmp",
                                        name=f"wt{h}_{kc}")
                        nc.scalar.dma_start(
                            wtmp[:],
                            Wd.ap().rearrange("(kc p) e -> p kc e", p=128)
                            [:, kc, h * 512:(h + 1) * 512])
                        nc.vector.tensor_copy(W_h[h][:, kc, :], wtmp[:])

            # fac2 = q @ U -> broadcast tiles fb[(b,h)] = [128,512] bf16
            # (U first: the fb chain is the longest pole for A00's first tanh)
            with (
                tc.tile_pool(name="f2u", bufs=1) as f2u,
                tc.tile_pool(name="f2", bufs=2) as f2p,
                tc.tile_pool(name="f2ps", bufs=2, space="PSUM") as f2ps,
            ):
                U_bf = f2u.tile([128, KC, D], BF16, name="U_bf")
                with tc.tile_pool(name="utp", bufs=2) as utp:
                    for kc in range(KC):
                        utmp = utp.tile([128, D], F32, tag="utmp",
                                        name=f"ut{kc}")
                        nc.scalar.dma_start(
                            utmp[:],
                            Ud.ap().rearrange("(kc p) e -> p kc e", p=128)[:, kc, :])
                        nc.vector.tensor_copy(U_bf[:, kc, :], utmp[:])
                fac2 = f2u.tile([1, BL, D], BF16, name="fac2")
                for b in range(BL):
                    qcol_f = f2p.tile([128, KC], F32, tag="qcf", name=f"qcf{b}")
                    nc.gpsimd.dma_start(
                        qcol_f[:], q2.ap()[b].rearrange("(kc p) -> p kc", p=128))
                    qcol = f2p.tile([128, KC], BF16, tag="qcb", name=f"qcb{b}")
                    nc.vector.tensor_copy(qcol[:], qcol_f[:])
                    for h in range(EH):
                        ps = f2ps.tile([1, 512], F32, tag="f2ps",
                                       name=f"f2ps{b}_{h}")
                        for kc in range(KC):
                            nc.tensor.matmul(ps[:], qcol[:, kc:kc + 1],
                                             U_bf[:, kc, h * 512:(h + 1) * 512],
                                             start=(kc == 0), stop=(kc == KC - 1))
                        nc.scalar.copy(fac2[0:1, b, h * 512:(h + 1) * 512], ps[:])
                # broadcast fac2 across partitions via K=1 matmul
                for b in range(BL):
                    for h in range(EH):
                        psb = f2ps.tile([128, 512], F32, tag="fbps",
                                        name=f"fbps{b}_{h}")
                        nc.tensor.matmul(psb[:], ones_bf[:],
                                         fac2[0:1, b, h * 512:(h + 1) * 512],
                                         start=True, stop=True)
                        fb[(b, h)] = wp.tile([128, 512], BF16, tag=f"fb{b}{h}",
                                             name=f"fb{b}_{h}")
                        nc.vector.tensor_copy(fb[(b, h)][:], psb[:])

            load_W_half(0)

            # warm up the collective machinery (after the setup DMAs so the
            # gpsimd queue isn't blocked while the barrier settles)
            ar_w1 = nc.gpsimd.collective_compute(
                "AllReduce", mybir.AluOpType.max, replica_groups=RG,
                ins=[warm_in.ap().opt()], outs=[warm_out.ap().opt()])
            ar_w2 = nc.gpsimd.collective_compute(
                "AllReduce", mybir.AluOpType.add, replica_groups=RG,
                ins=[warm_out.ap().opt()], outs=[warm_out2.ap().opt()])

            # rotating load pools
            vp_cm = tc.tile_pool(name="vp", bufs=2)
            vp = vp_cm.__enter__()
            vbp_cm = tc.tile_pool(name="vbp", bufs=3)
            vbp = vbp_cm.__enter__()
            kp_cm = tc.tile_pool(name="kp", bufs=2)
            kp = kp_cm.__enter__()

            s_cur = [None] * BL
            s_h = {}
            attn = {}
            vT = {0: [], 1: []}

            def stage_A(b, h):
                """returns (t_pool_cm, t_tile); caller's stage_B closes it"""
                t_cm = tc.tile_pool(name=f"t{b}{h}", bufs=1)
                tp = t_cm.__enter__()
                t_bh = tp.tile([128, M_T, 512], BF16, name=f"t{b}_{h}")
                with (
                    tc.tile_pool(name=f"ktm{b}{h}", bufs=2) as ktp,
                    tc.tile_pool(name=f"A{b}{h}ps", bufs=3, space="PSUM") as aps,
                    tc.tile_pool(name=f"A{b}{h}tp", bufs=2, space="PSUM") as tps,
                ):
                    for m in range(M_T):
                        ktm = ktp.tile([128, KC, 128], BF16, tag="ktm",
                                       name=f"ktm{b}_{h}_{m}")
                        if h == 0:
                            kslab = kp.tile([128, D], F32, tag="kslab",
                                            name=f"kslab{b}_{m}")
                            nc.sync.dma_start(
                                kslab[:], k2.ap()[b, m * 128:(m + 1) * 128, :])
                            for g in range(2):
                                ptr = tps.tile([128, 512], F32, tag="ptr",
                                               name=f"ptr{b}_{m}_{g}")
                                for i in range(4):
                                    kc = g * 4 + i
                                    nc.tensor.transpose(
                                        ptr[:, i * 128:(i + 1) * 128],
                                        kslab[:, kc * 128:(kc + 1) * 128],
                                        ident[:])
                                nc.vector.tensor_copy(
                                    ktm[:, g * 4:(g + 1) * 4, :], ptr[:])
                            nc.sync.dma_start(kt_d.ap()[b][:, m, :, :], ktm[:])
                        else:
                            nc.sync.dma_start(ktm[:], kt_d.ap()[b][:, m, :, :])
                        ps = aps.tile([128, 512], F32, tag="aps",
                                      name=f"aps{b}_{h}_{m}")
                        for kc in range(KC):
                            nc.tensor.matmul(
                                ps[:], ktm[:, kc, :], W_h[h][:, kc, :],
                                start=(kc == 0), stop=(kc == KC - 1))
                        nc.vector.tensor_add(ps[:], ps[:], fb[(b, h)][:])
                        nc.scalar.activation(t_bh[:, m, :], ps[:], AF.Tanh)
                return t_cm, t_bh

            def stage_B(b, h, t_cm, t_bh, vf_list=None):
                s_t = spool.tile([128, KC, 512], F32, tag=f"s{b}",
                                 name=f"s{b}_{h}")
                s_cur[b] = s_t
                with tc.tile_pool(name=f"B{b}{h}ps", bufs=1,
                                  space="PSUM") as bps:
                    psb = [bps.tile([128, 512], F32, tag=f"pb{dt}",
                                    name=f"pb{b}_{h}_{dt}") for dt in range(KC)]
                    for m in range(M_T):
                        if vf_list is not None:
                            vf = vf_list[m]
                        else:
                            vf = vp.tile([128, D], F32, tag="vf",
                                         name=f"vf{b}_{h}_{m}")
                            nc.scalar.dma_start(
                                vf[:], v2.ap()[b, m * 128:(m + 1) * 128, :])
                        vb = vbp.tile([128, D], BF16, tag="vb",
                                      name=f"vbb{b}_{h}_{m}")
                        nc.vector.tensor_copy(vb[:], vf[:])
                        if h == 1:
                            vt = vtp.tile([128, KC, 128], BF16,
                                          tag=f"vt{b}_{m}", name=f"vt{b}_{m}")
                            nc.sync.dma_start(vt[:], vb[:], transpose=True)
                            vT[b].append(vt)
                        for dt in range(KC):
                            nc.tensor.matmul(
                                psb[dt][:],
                                vb[:, dt * 128:(dt + 1) * 128],
                                t_bh[:, m, :],
                                start=(m == 0), stop=(m == M_T - 1))
                    for dt in range(KC):
                        nc.vector.tensor_copy(s_t[:, dt, :], psb[dt][:])
                t_cm.__exit__(None, None, None)

            prev_ar = [ar_w2]

            def sm_max(h):
                s_h[h] = list(s_cur)
                for b in range(BL):
                    attn[(b, h)] = attnp.tile([128, KC, 512], BF16,
                                              tag=f"at{b}{h}",
                                              name=f"attn{b}_{h}")
                for c in range(2 * ARC):
                    dsl = slice(c, c + 1)
                    mx = smp.tile([128, 1, 512], BF16, tag="bc1",
                                  name=f"mx{h}_{c}")
                    nc.vector.tensor_max(mx[:], s_h[h][0][:, dsl, :],
                                         s_h[h][1][:, dsl, :])
                    nc.gpsimd.dma_start(mx_in[h].ap()[:, dsl, :], mx[:])
                ar_mx = nc.gpsimd.collective_compute(
                    "AllReduce", mybir.AluOpType.max, replica_groups=RG,
                    ins=[mx_in[h].ap().opt()], outs=[mx_out[h].ap().opt()])
                tile.add_dep_helper(ar_mx.ins, prev_ar[0].ins, sync=False,
                                    reason="serialize collectives")
                prev_ar[0] = ar_mx

            def sm_exp(h, eng):
                for c in range(2 * ARC):
                    dsl = slice(c, c + 1)
                    gmxb = smp.tile([128, 1, 512], BF16, tag="bc2",
                                    name=f"gmxb{h}_{c}")
                    nc.gpsimd.dma_start(gmxb[:], mx_out[h].ap()[:, dsl, :])
                    for b in range(BL):
                        eng.tensor_sub(s_h[h][b][:, dsl, :],
                                       s_h[h][b][:, dsl, :], gmxb[:])
                        nc.scalar.activation(attn[(b, h)][:, dsl, :],
                                             s_h[h][b][:, dsl, :], AF.Exp)
                    sm = smp.tile([128, 1, 512], BF16, tag="bc1",
                                  name=f"sm{h}_{c}")
                    eng.tensor_add(sm[:], attn[(0, h)][:, dsl, :],
                                   attn[(1, h)][:, dsl, :])
                    nc.gpsimd.dma_start(sm_in[h].ap()[:, dsl, :], sm[:])
                ar_sm = nc.gpsimd.collective_compute(
                    "AllReduce", mybir.AluOpType.add, replica_groups=RG,
                    ins=[sm_in[h].ap().opt()], outs=[sm_out[h].ap().opt()])
                tile.add_dep_helper(ar_sm.ins, prev_ar[0].ins, sync=False,
                                    reason="serialize collectives")
                prev_ar[0] = ar_sm

            def sm_rec(h):
                # rec = 1/Z via fast approx (Z >= 1, so no edge cases);
                # attn = p * rec in place (bf16)
                for c in range(2 * ARC):
                    dsl = slice(c, c + 1)
                    zz = smp.tile([128, 1, 512], BF16, tag="bc2",
                                  name=f"zz{h}_{c}")
                    nc.gpsimd.dma_start(zz[:], sm_out[h].ap()[:, dsl, :])
                    zf = smp.tile([128, 1, 512], F32, tag="zf",
                                  name=f"zf{h}_{c}")
                    nc.vector.tensor_copy(zf[:], zz[:])
                    rec = smp.tile([128, 1, 512], F32, tag="rec",
                                   name=f"rec{h}_{c}")
                    nc.vector.reciprocal_approx_fast(rec[:], zf[:])
                    for b in range(BL):
                        nc.vector.tensor_mul(attn[(b, h)][:, dsl, :],
                                             attn[(b, h)][:, dsl, :], rec[:])

            # ======== main schedule ========
            t_cm, t_bh = stage_A(0, 0)
            stage_B(0, 0, t_cm, t_bh)
            t_cm, t_bh = stage_A(1, 0)
            load_W_half(1)          # rotate W to the h1 half during A10/B10
            stage_B(1, 0, t_cm, t_bh)
            kp_cm.__exit__(None, None, None)

            sm_max(0)
            t_cm0, t_bh0 = stage_A(0, 1)
            t_cm1, t_bh1 = stage_A(1, 1)
            # pre-emit the h1 v loads so their DMA-completion semaphores are
            # allocated BEFORE the AR-gated bounce DMAs (avoids the shared-
            # semaphore convoy that stalls B11/B01 behind the sm0 chain).
            # B11's go on sync, B01's on scalar: separate rings, so a not-
            # yet-ready descriptor never blocks the other stage's stream.
            vfs1 = []
            for m in range(M_T):
                vf = vp.tile([128, D], F32, tag="vf", name=f"vf1_1_{m}")
                nc.sync.dma_start(vf[:], v2.ap()[1, m * 128:(m + 1) * 128, :])
                vfs1.append(vf)
            vfs0 = []
            for m in range(M_T):
                vf = vp.tile([128, D], F32, tag="vf", name=f"vf0_1_{m}")
                nc.scalar.dma_start(vf[:], v2.ap()[0, m * 128:(m + 1) * 128, :])
                vfs0.append(vf)
            sm_exp(0, nc.gpsimd)
            stage_B(1, 1, t_cm1, t_bh1, vf_list=vfs1)
            stage_B(0, 1, t_cm0, t_bh0, vf_list=vfs0)
            sm_rec(0)

            vbp_cm.__exit__(None, None, None)
            vp_cm.__exit__(None, None, None)

            sm_max(1)

            # ======== stage C (interleaved with softmax h1 tail) ========
            cps_cm = tc.tile_pool(name="cps", bufs=6, space="PSUM")
            cps = cps_cm.__enter__()

            def stage_c(b, h):
                he = slice(h * 512, (h + 1) * 512)
                for m in range(M_T):
                    ps = cps.tile([128, 512], F32, tag="cps",
                                  name=f"cps{b}_{h}_{m}")
                    for kc in range(KC):
                        nc.tensor.matmul(
                            ps[:], vT[b][m][:, kc, :],
                            attn[(b, h)][:, kc, :],
                            start=(kc == 0), stop=(kc == KC - 1))
                    ost = cpool.tile([128, 512], F32, tag="ost",
                                     name=f"ost{b}_{h}_{m}")
                    nc.scalar.copy(ost[:], ps[:])
                    nc.sync.dma_start(
                        out2.ap()[b, m * 128:(m + 1) * 128, he], ost[:])

            stage_c(0, 0)
            sm_exp(1, nc.vector)
            stage_c(1, 0)
            sm_rec(1)

            wp_cm.__exit__(None, None, None)
            sp_cm.__exit__(None, None, None)
            sm_cm.__exit__(None, None, None)

            stage_c(0, 1)
            stage_c(1, 1)

            cps_cm.__exit__(None, None, None)
            ap_cm.__exit__(None, None, None)
            vt_cm.__exit__(None, None, None)
            cp_cm.__exit__(None, None, None)

    nc.compile()
    return nc


_NC = None


def _get_nc():
    global _NC
    if _NC is None:
        _NC = build()
    return _NC


def kernel(q, k, v, W, U):
    q = np.ascontiguousarray(np.asarray(q, dtype=np.float32))
    k = np.ascontiguousarray(np.asarray(k, dtype=np.float32))
    v = np.ascontiguousarray(np.asarray(v, dtype=np.float32))
    W = np.ascontiguousarray(np.asarray(W, dtype=np.float32))
    U = np.ascontiguousarray(np.asarray(U, dtype=np.float32))

    nc = _get_nc()
    in_maps = [
        {
            "q2": q[c * BL:(c + 1) * BL],
            "k2": k[c * BL:(c + 1) * BL],
            "v2": v[c * BL:(c + 1) * BL],
            "W": W,
            "U": U,
        }
        for c in range(N_CORES)
    ]
    res = run_bass_kernel_spmd(nc, in_maps, core_ids=list(range(N_CORES)))
    out = np.concatenate([res.results[c]["out"] for c in range(N_CORES)], axis=0)
    return out.astype(np.float32)


if __name__ == "__main__":
    rng = np.random.default_rng(0)
    q = rng.standard_normal((B, D), dtype=np.float32)
    k = rng.standard_normal((B, S, D), dtype=np.float32)
    v = rng.standard_normal((B, S, D), dtype=np.float32)
    W = (rng.standard_normal((D, D), dtype=np.float32) / np.sqrt(D)).astype(np.float32)
    U = (rng.standard_normal((D, D), dtype=np.float32) / np.sqrt(D)).astype(np.float32)
    out = kernel(q=q, k=k, v=v, W=W, U=U)
    print("out", out.shape, out.dtype, float(np.abs(out).mean()))
